# revision 2
# baseline (speedup 1.0000x reference)
"""Trainium2 Bass kernel for nn_BiLSTMNet — transposed-gates formulation.

Key idea vs baseline: the recurrent/gate matmuls are emitted with the GATE
dimension on PSUM partitions and the (step, lane) batch on the free dim, so
each matmul instruction's cost (= out free size in this cost model) is 16-64
rows instead of 800.  Gate blocks are padded to 256 rows (M=1024, 8 chunks of
128) so sigmoid/tanh and the DVE cell update run as a handful of wide
instructions per step.  h^T is produced directly in matmul-rhs layout (no
per-step transposes), h history is buffered in SBUF for 64 steps and stored
with 2 DMAs per group (HWDGE count ~200 vs ~8000 in the baseline).
The g-gate weights are pre-scaled by 2 so a single sigmoid covers all four
gates (tanh(g) = 2*sigmoid(2g) - 1).
"""
import sys
sys.path.insert(0, "/opt/trn_rl_repo")
import numpy as np
import ml_dtypes

import concourse.bass as bass
import concourse.tile as tile
from concourse import mybir, bacc
from concourse.bass_utils import run_bass_kernel_spmd
from concourse.masks import make_identity

BF16 = mybir.dt.bfloat16
F32 = mybir.dt.float32
I32 = mybir.dt.int32
AF = mybir.ActivationFunctionType
ALU = mybir.AluOpType

V, E, H, B, C = 32000, 200, 200, 128, 256
T = 512
BL = 16            # sentences per core
NCORE = 8
CHT = 4            # timesteps per chunk
NCH = T // CHT     # 128 chunks
GCH = 16           # chunks per h-store group
GSTEP = GCH * CHT  # 64 steps per group
NGRP = NCH // GCH  # 8 groups
NSLOT = T * BL     # 8192
M4 = 1024          # gate-padded M (4 gates x 256)


def build(NPT):
    nc = bacc.Bacc("TRN2", target_bir_lowering=False, debug=False,
                   enable_asserts=True, num_devices=NCORE)

    def din(name, shape, dt):
        return nc.dram_tensor(name, shape, dt, kind="ExternalInput").ap()

    def dout(name, shape, dt):
        return nc.dram_tensor(name, shape, dt, kind="ExternalOutput").ap()

    emb = din("emb", [V, E], BF16)
    W0T = [din(f"W0T{d}", [256, M4], BF16) for d in range(2)]
    Whh0T = [din(f"Whh0T{d}", [200, M4], BF16) for d in range(2)]
    W1T = [din(f"W1T{d}", [401, M4], BF16) for d in range(2)]
    Whh1T = [din(f"Whh1T{d}", [200, M4], BF16) for d in range(2)]
    WUs = din("WUs", [401, 800], BF16)
    W2s = din("W2s", [512, 4], BF16)
    tokf = din("tokf", [CHT * BL, NCH], I32)   # [slot(st,lane), chunk]
    tokb = din("tokb", [CHT * BL, NCH], I32)
    uidx0 = din("uidx0", [128, NPT], I32)
    uidx1 = din("uidx1", [128, NPT], I32)
    umask0 = din("umask0", [128, NPT], F32)
    umask1 = din("umask1", [128, NPT], F32)
    bw1m = din("bw1m", [128, 2 * H], BF16)

    OUT = dout("OUT", [NPT * 128, 4], F32)

    # internal DRAM: h^T in t-major order; rows 0:200 fwd, 200:400 bwd, 400 ones
    h0T = nc.dram_tensor("h0T", [401, NSLOT], BF16).ap()
    h1T = nc.dram_tensor("h1T", [401, NSLOT], BF16).ap()
    U0d = nc.dram_tensor("U0d", [NSLOT, 2 * H], BF16).ap()
    U1d = nc.dram_tensor("U1d", [NSLOT, 2 * H], BF16).ap()

    with tile.TileContext(nc) as tc:
        with tc.tile_pool(name="const", bufs=1) as cp, \
             tc.tile_pool(name="state", bufs=1) as sp:

            def load_tiles(src, rows, ncols, pref):
                tiles = []
                r0 = 0
                for h_ in rows:
                    t_ = cp.tile([h_, ncols], BF16, tag=f"{pref}{r0}",
                                 name=f"{pref}{r0}")
                    nc.sync.dma_start(out=t_[:], in_=src[r0:r0 + h_, :])
                    tiles.append(t_)
                    r0 += h_
                return tiles

            W0t = [load_tiles(W0T[d], [128, 128], M4, f"w0{d}") for d in range(2)]
            Whh0t = [load_tiles(Whh0T[d], [128, 72], M4, f"wh0{d}") for d in range(2)]
            W1t = [load_tiles(W1T[d], [128, 128, 128, 17], M4, f"w1{d}") for d in range(2)]
            Whh1t = [load_tiles(Whh1T[d], [128, 72], M4, f"wh1{d}") for d in range(2)]
            WUt = load_tiles(WUs, [128, 128, 128, 17], 800, "wu")
            W2t = load_tiles(W2s, [128, 128, 128, 128], 4, "w2")

            tok_t = [cp.tile([CHT * BL, NCH], I32, tag=f"tok{d}", name=f"tok{d}")
                     for d in range(2)]
            nc.sync.dma_start(out=tok_t[0][:], in_=tokf[:])
            nc.sync.dma_start(out=tok_t[1][:], in_=tokb[:])

            ones_row = cp.tile([1, NSLOT], BF16, name="ones_row")
            nc.vector.memset(ones_row[:], 1.0)
            nc.sync.dma_start(out=h0T[400:401, :], in_=ones_row[:])
            nc.sync.dma_start(out=h1T[400:401, :], in_=ones_row[:])

            ident64 = sp.tile([64, 64], BF16, name="ident64")
            make_identity(nc, ident64[:])
            ident128 = sp.tile([128, 128], BF16, name="ident128")
            make_identity(nc, ident128[:])

            # persistent LSTM state
            gx = [[sp.tile([CHT * BL, 256], BF16, tag=f"gx{d}{p}", name=f"gx{d}{p}")
                   for p in range(2)] for d in range(2)]
            for d in range(2):
                for p in range(2):
                    nc.vector.memset(gx[d][p][:], 0.0)
                    nc.vector.memset(gx[d][p][:, 255:256], 1.0)
            Hh = [[sp.tile([128, GSTEP * 32], BF16, tag=f"Hh{d}{p}", name=f"Hh{d}{p}")
                   for p in range(2)] for d in range(2)]
            cS = [sp.tile([128, 32], F32, tag=f"cS{d}", name=f"cS{d}")
                  for d in range(2)]
            onesb = sp.tile([128, 32], F32, name="onesb")
            nc.vector.memset(onesb[:], 1.0)

            def hslot(d, p):
                return (p % GSTEP) if d == 0 else (GSTEP - 1 - (p % GSTEP))

            def hpar(p):
                return (p // GSTEP) % 2

            # ============ LSTM layers ============
            with tc.tile_pool(name="work", bufs=2) as wp, \
                 tc.tile_pool(name="pg", bufs=2, space="PSUM") as pgp, \
                 tc.tile_pool(name="xp", bufs=2, space="PSUM") as xpp, \
                 tc.tile_pool(name="rhs1", bufs=2) as rp, \
                 tc.tile_pool(name="uw", bufs=2) as uw, \
                 tc.tile_pool(name="ub", bufs=2) as ubp, \
                 tc.tile_pool(name="ups", bufs=1, space="PSUM") as ups:

                xparts = {}
                pg = {}
                rhs1 = {}

                def emit_x0(k):
                    par = k % 2
                    ps = xpp.tile([128, 256], BF16, space="PSUM",
                                  tag="xps", name="xps",
                                  padded_shape=[128, 1024])
                    for d in range(2):
                        gxt = gx[d][par]
                        nc.gpsimd.indirect_dma_start(
                            out=gxt[:, 0:E], out_offset=None, in_=emb[:],
                            in_offset=bass.IndirectOffsetOnAxis(
                                ap=tok_t[d][:, k:k + 1], axis=0))
                        for half in range(2):
                            c0 = (2 * d + half) * 64
                            nc.tensor.transpose(
                                ps[:, c0:c0 + 64],
                                gxt[:, half * 128:(half + 1) * 128],
                                ident64[:])
                            xt = wp.tile([128, 64], BF16, tag=f"x{d}{half}",
                                         name=f"x{d}{half}")
                            nc.vector.tensor_copy(xt[:], ps[:, c0:c0 + 64])
                            xparts[(k, d, half)] = xt

                def emit_xg0(k):
                    for d in range(2):
                        pgt = pgp.tile([128, 512], F32, space="PSUM",
                                       tag=f"PG{d}", name=f"PG{d}")
                        pg[(k, d)] = pgt
                        for kc in range(2):
                            rhs = xparts.pop((k, d, kc))
                            for mc in range(8):
                                nc.tensor.matmul(
                                    pgt[:, mc * 64:(mc + 1) * 64],
                                    W0t[d][kc][:, mc * 128:(mc + 1) * 128],
                                    rhs[:], start=(kc == 0 and mc == 0),
                                    stop=False, skip_group_check=True)

                def load_rhs1(g):
                    for d in range(2):
                        col0 = g * GSTEP * BL if d == 0 else (T - GSTEP * (g + 1)) * BL
                        tiles = []
                        for (r0, r1) in ((0, 128), (128, 256), (256, 384), (384, 401)):
                            t_ = rp.tile([r1 - r0, GSTEP * BL], BF16,
                                         tag=f"R{d}{r0}", name=f"R{d}{r0}")
                            nc.sync.dma_start(
                                out=t_[:], in_=h0T[r0:r1, col0:col0 + GSTEP * BL])
                            tiles.append(t_)
                        rhs1[(g, d)] = tiles

                def emit_xg1(k):
                    g, cg = k // GCH, k % GCH
                    for d in range(2):
                        pgt = pgp.tile([128, 512], F32, space="PSUM",
                                       tag=f"PG{d}", name=f"PG{d}")
                        pg[(k, d)] = pgt
                        tiles = rhs1[(g, d)]
                        coff = cg * 64 if d == 0 else (GSTEP - 4 - 4 * cg) * BL
                        for kc in range(4):
                            rhs = tiles[kc][:, coff:coff + 64]
                            for mc in range(8):
                                nc.tensor.matmul(
                                    pgt[:, mc * 64:(mc + 1) * 64],
                                    W1t[d][kc][:, mc * 128:(mc + 1) * 128],
                                    rhs, start=(kc == 0 and mc == 0),
                                    stop=False, skip_group_check=True)

                def emit_step_dir(k, st, d, Whht, layer):
                    # full per-direction step sequence; f/b emitted alternately
                    # so the two chains phase-shift on the in-order queues
                    p = CHT * k + st
                    q = st if (d == 0 or layer == 0) else CHT - 1 - st
                    pgt = pg[(k, d)]
                    hs = hslot(d, p - 1)
                    hprev = Hh[d][hpar(p - 1)]
                    for kc in range(2):
                        if kc == 0:
                            rhs = hprev[:, hs * 32:hs * 32 + 16]
                        else:
                            rhs = hprev[0:72, hs * 32 + 16:hs * 32 + 32]
                        for mc in range(8):
                            nc.tensor.matmul(
                                pgt[:, mc * 64 + q * 16:mc * 64 + q * 16 + 16],
                                Whht[d][kc][:, mc * 128:(mc + 1) * 128],
                                rhs, start=False, stop=(kc == 1),
                                skip_group_check=True)
                    G = wp.tile([128, 128], F32, tag=f"G{d}", name=f"G{d}")
                    nc.scalar.activation(
                        G[:].rearrange("p (m s) -> p m s", s=16),
                        pgt[:].rearrange("p (m s) -> p m s", s=64)[:, :, q * 16:(q + 1) * 16],
                        AF.Sigmoid)
                    eng = nc.vector
                    dg = wp.tile([128, 32], F32, tag=f"d{d}", name=f"d{d}")
                    eng.scalar_tensor_tensor(
                        dg[:], G[:, 96:128], 2.0, onesb[:], ALU.mult, ALU.subtract)
                    ag = wp.tile([128, 32], F32, tag=f"a{d}", name=f"a{d}")
                    eng.scalar_tensor_tensor(
                        ag[:], G[:, 32:64], 1.0, dg[:], ALU.mult, ALU.mult)
                    Xg = wp.tile([128, 32], F32, tag=f"X{d}", name=f"X{d}")
                    nc.vector.scalar_tensor_tensor(
                        Xg[:], G[:, 0:32], 1.0, cS[d][:], ALU.mult, ALU.mult)
                    nc.vector.scalar_tensor_tensor(
                        cS[d][:], ag[:], 1.0, Xg[:], ALU.mult, ALU.add)
                    TC = wp.tile([128, 32], F32, tag=f"tc{d}", name=f"tc{d}")
                    nc.scalar.activation(TC[:], cS[d][:], AF.Tanh)
                    hsl = hslot(d, p)
                    nc.vector.scalar_tensor_tensor(
                        Hh[d][hpar(p)][:, hsl * 32:(hsl + 1) * 32],
                        G[:, 64:96], 1.0, TC[:], ALU.mult, ALU.mult)

                def emit_step(k, st, Whht, layer):
                    emit_step_dir(k, st, 0, Whht, layer)
                    emit_step_dir(k, st, 1, Whht, layer)

                def store_h(layer, g):
                    hT = h0T if layer == 0 else h1T
                    par = g % 2
                    for d in range(2):
                        rb = 0 if d == 0 else 200
                        tcol0 = g * GSTEP * BL if d == 0 else (T - GSTEP * (g + 1)) * BL
                        src = Hh[d][par]
                        v = src[:].rearrange("p (s c) -> p s c", c=32)
                        v72 = src[0:72, :].rearrange("p (s c) -> p s c", c=32)
                        nc.sync.dma_start(
                            out=hT[rb:rb + 128, tcol0:tcol0 + GSTEP * BL],
                            in_=v[:, :, 0:16])
                        nc.sync.dma_start(
                            out=hT[rb + 128:rb + 200, tcol0:tcol0 + GSTEP * BL],
                            in_=v72[:, :, 16:32])

                # ---- U phase machinery (interleaved into L1's latency gaps)
                UGC = 8                 # chunks (of 128 slots) per U group
                ustate = {"cur": None, "c8": 0, "lt": None, "Ub": None}
                uready = []

                def u_load_group(ug):
                    c0 = ug * UGC * 128
                    lt = []
                    for (r0, r1) in ((0, 128), (128, 256), (256, 384), (384, 401)):
                        t_ = uw.tile([r1 - r0, UGC * 128], BF16, tag=f"ul{r0}",
                                     name=f"ul{r0}")
                        nc.sync.dma_start(out=t_[:], in_=h1T[r0:r1, c0:c0 + UGC * 128])
                        lt.append(t_)
                    Ub = [ubp.tile([128, UGC * 400], BF16, tag=f"Ub{i}",
                                   name=f"Ub{i}") for i in range(2)]
                    ustate.update(cur=ug, c8=0, lt=lt, Ub=Ub)

                def u_emit_chunk():
                    ug, c8 = ustate["cur"], ustate["c8"]
                    lt, Ub = ustate["lt"], ustate["Ub"]
                    psu = ups.tile([128, 800], F32, space="PSUM", tag="psu",
                                   name="psu")
                    for kc in range(4):
                        for ns in range(7):
                            n0, n1 = ns * 128, min((ns + 1) * 128, 800)
                            nc.tensor.matmul(
                                psu[:, n0:n1],
                                lt[kc][:, c8 * 128:(c8 + 1) * 128],
                                WUt[kc][:, n0:n1],
                                start=(kc == 0 and ns in (0, 4)),
                                stop=(kc == 3), skip_group_check=True)
                    nc.vector.tensor_copy(Ub[0][:, c8 * 400:c8 * 400 + 200],
                                          psu[:, 0:200])
                    nc.vector.tensor_copy(Ub[0][:, c8 * 400 + 200:(c8 + 1) * 400],
                                          psu[:, 200:400])
                    nc.scalar.copy(Ub[1][:, c8 * 400:(c8 + 1) * 400],
                                   psu[:, 400:800])
                    if c8 == UGC - 1:
                        c0 = ug * UGC * 128
                        for i, Ud in enumerate((U0d, U1d)):
                            dst = Ud[c0:c0 + UGC * 128, :].rearrange(
                                "(c p) e -> p c e", p=128)
                            nc.sync.dma_start(
                                out=dst,
                                in_=Ub[i][:].rearrange("p (c e) -> p c e", e=400))
                        ustate["cur"] = None
                    else:
                        ustate["c8"] = c8 + 1

                def u_pump(n):
                    for _ in range(n):
                        if ustate["cur"] is None:
                            if not uready:
                                return
                            u_load_group(uready.pop(0))
                        u_emit_chunk()

                def run_layer(layer):
                    Whht = Whh0t if layer == 0 else Whh1t
                    for d in range(2):
                        nc.vector.memset(cS[d][:], 0.0)
                    # zero the h_prev slots read at p=0 (python-mod indices)
                    nc.vector.memset(
                        Hh[0][hpar(-1)][:, hslot(0, -1) * 32:(hslot(0, -1) + 1) * 32], 0.0)
                    nc.vector.memset(
                        Hh[1][hpar(-1)][:, hslot(1, -1) * 32:(hslot(1, -1) + 1) * 32], 0.0)
                    if layer == 0:
                        emit_x0(0)
                        emit_xg0(0)
                    else:
                        load_rhs1(0)
                        emit_xg1(0)
                    for k in range(NCH):
                        if layer == 1 and k % GCH == 8 and k // GCH + 1 < NGRP:
                            load_rhs1(k // GCH + 1)
                        if layer == 0 and k + 1 < NCH:
                            emit_x0(k + 1)
                        emit_step(k, 0, Whht, layer)
                        emit_step(k, 1, Whht, layer)
                        if k + 1 < NCH:
                            if layer == 0:
                                emit_xg0(k + 1)
                            else:
                                emit_xg1(k + 1)
                        emit_step(k, 2, Whht, layer)
                        emit_step(k, 3, Whht, layer)
                        if layer == 1:
                            u_pump(1)
                        if k % GCH == GCH - 1:
                            g = k // GCH
                            store_h(layer, g)
                            if layer == 1 and g >= NGRP // 2:
                                # U group ug needs L1-f group ug and L1-b group
                                # NGRP-1-ug; both done once g >= max(ug, 7-ug)
                                if g == NGRP - 1:
                                    uready.extend([0, NGRP - 1])
                                else:
                                    uready.extend([NGRP - 1 - g, g])

                run_layer(0)
                run_layer(1)
                # drain remaining U work
                u_pump(NGRP * UGC)

            # ============ gather + MLP ============
            with tc.tile_pool(name="fw", bufs=3) as fw, \
                 tc.tile_pool(name="fc", bufs=1) as fc, \
                 tc.tile_pool(name="ob", bufs=2) as obp, \
                 tc.tile_pool(name="fps", bufs=2, space="PSUM") as fps:
                ui0 = fc.tile([128, NPT], I32, name="ui0")
                ui1 = fc.tile([128, NPT], I32, name="ui1")
                um0 = fc.tile([128, NPT], F32, name="um0")
                um1 = fc.tile([128, NPT], F32, name="um1")
                nc.sync.dma_start(out=ui0[:], in_=uidx0[:])
                nc.sync.dma_start(out=ui1[:], in_=uidx1[:])
                nc.sync.dma_start(out=um0[:], in_=umask0[:])
                nc.sync.dma_start(out=um1[:], in_=umask1[:])
                bwt = fc.tile([128, 2 * H], BF16, name="bwt")
                nc.sync.dma_start(out=bwt[:], in_=bw1m[:])
                hm = [fc.tile([128, 512], BF16, tag=f"hm{i}", name=f"hm{i}")
                      for i in range(2)]
                for t_ in hm:
                    nc.vector.memset(t_[:], 0.0)
                    nc.vector.memset(t_[:, 511:512], 1.0)
                OBW = 8
                for j in range(NPT):
                    par = j % 2
                    if j % OBW == 0:
                        ob = obp.tile([128, OBW * 4], F32, tag="ob", name="ob")
                    g0 = fw.tile([128, 2 * H], BF16, tag="g0", name="g0")
                    g1 = fw.tile([128, 2 * H], BF16, tag="g1", name="g1")
                    nc.gpsimd.indirect_dma_start(
                        out=g0[:], out_offset=None, in_=U0d[:],
                        in_offset=bass.IndirectOffsetOnAxis(ap=ui0[:, j:j + 1], axis=0))
                    nc.gpsimd.indirect_dma_start(
                        out=g1[:], out_offset=None, in_=U1d[:],
                        in_offset=bass.IndirectOffsetOnAxis(ap=ui1[:, j:j + 1], axis=0))
                    g1m = fw.tile([128, 2 * H], BF16, tag="g1m", name="g1m")
                    nc.vector.scalar_tensor_tensor(
                        g1m[:], g1[:], um1[:, j:j + 1], bwt[:], ALU.mult, ALU.add)
                    ssum = fw.tile([128, 2 * H], BF16, tag="ssum", name="ssum")
                    nc.vector.scalar_tensor_tensor(
                        ssum[:], g0[:], um0[:, j:j + 1], g1m[:], ALU.mult, ALU.add)
                    nc.scalar.activation(hm[par][:, 0:2 * H], ssum[:], AF.Tanh)
                    hmT = []
                    pst = fps.tile([128, 512], BF16, space="PSUM",
                                   tag="pst", name="pst",
                                   padded_shape=[128, 1024])
                    for i in range(4):
                        nc.tensor.transpose(
                            pst[:, i * 128:(i + 1) * 128],
                            hm[par][:, i * 128:(i + 1) * 128], ident128[:])
                        ht_ = fw.tile([128, 128], BF16, tag=f"hmT{i}", name=f"hmT{i}")
                        nc.vector.tensor_copy(ht_[:], pst[:, i * 128:(i + 1) * 128])
                        hmT.append(ht_)
                    psl = fps.tile([128, 4], F32, space="PSUM", tag="psl",
                                   name="psl", padded_shape=[128, 512])
                    for i in range(4):
                        nc.tensor.matmul(psl[:], hmT[i][:], W2t[i][:],
                                         start=(i == 0), stop=(i == 3))
                    ex = fw.tile([128, 4], F32, tag="ex", name="ex")
                    nc.scalar.activation(ex[:], psl[:], AF.Exp)
                    sm = fw.tile([128, 1], F32, tag="sm", name="sm")
                    nc.vector.reduce_sum(sm[:], ex[:], axis=mybir.AxisListType.X)
                    rc = fw.tile([128, 1], F32, tag="rc", name="rc")
                    nc.vector.reciprocal(rc[:], sm[:])
                    nc.vector.tensor_scalar_mul(
                        ob[:, (j % OBW) * 4:(j % OBW + 1) * 4], ex[:], rc[:, 0:1])
                    if j % OBW == OBW - 1:
                        j0 = j - OBW + 1
                        dst = OUT[j0 * 128:(j0 + OBW) * 128, :].rearrange(
                            "(a p) e -> p a e", p=128)
                        nc.sync.dma_start(
                            out=dst,
                            in_=ob[:].rearrange("p (a e) -> p a e", e=4))
    nc.compile()
    return nc


# ---------------------------------------------------------------------------
# host-side preparation
# ---------------------------------------------------------------------------

def _perm_scale(w):
    """torch gate order (i,f,g,o) -> (f,i,o,g) along axis 0; scale g block x2."""
    Hq = w.shape[0] // 4
    i, f, g, o = (w[0:Hq], w[Hq:2 * Hq], w[2 * Hq:3 * Hq], w[3 * Hq:4 * Hq])
    return np.concatenate([f, i, o, 2.0 * g], axis=0)


def _gatepad(wT):
    """[K, 800] -> [K, 1024] with each 200-row gate block padded to 256."""
    K = wT.shape[0]
    out = np.zeros((K, M4), np.float32)
    for gb in range(4):
        out[:, gb * 256:gb * 256 + 200] = wT[:, gb * 200:(gb + 1) * 200]
    return out


def prepare_inputs(inputs):
    bf = ml_dtypes.bfloat16
    C_ = np.asarray(inputs["confs"]).shape[1]
    emb = np.asarray(inputs["emb"], np.float32)
    tokens = np.asarray(inputs["tokens"])
    confs = np.asarray(inputs["confs"])

    p = {}
    p["emb"] = emb.astype(bf)

    def prep_dir(Wih, Whh, b, kin, kpad, name_w, name_h):
        Wp = _perm_scale(np.asarray(Wih, np.float32))      # [800, kin]
        bp = _perm_scale(np.asarray(b, np.float32))        # [800]
        Hp = _perm_scale(np.asarray(Whh, np.float32))      # [800, 200]
        wt = np.zeros((kpad, M4), np.float32)
        wt[0:kin] = _gatepad(Wp.T)
        wt[kpad - 1] = _gatepad(bp[None, :])[0]
        p[name_w] = wt.astype(bf)
        p[name_h] = _gatepad(Hp.T).astype(bf)

    prep_dir(inputs["Wih0f"], inputs["Whh0f"], inputs["b0f"], E, 256, "W0T0", "Whh0T0")
    prep_dir(inputs["Wih0b"], inputs["Whh0b"], inputs["b0b"], E, 256, "W0T1", "Whh0T1")
    prep_dir(inputs["Wih1f"], inputs["Whh1f"], inputs["b1f"], 400, 401, "W1T0", "Whh1T0")
    prep_dir(inputs["Wih1b"], inputs["Whh1b"], inputs["b1b"], 400, 401, "W1T1", "Whh1T1")

    w1 = np.asarray(inputs["w1"], np.float32)
    bw1 = np.asarray(inputs["bw1"], np.float32)
    w2 = np.asarray(inputs["w2"], np.float32)
    bw2 = np.asarray(inputs["bw2"], np.float32)

    wu = np.zeros((401, 800), np.float32)
    wu[0:400, 0:400] = w1[:, 0:400].T
    wu[0:400, 400:800] = w1[:, 400:800].T
    p["WUs"] = wu.astype(bf)
    p["bw1m"] = np.tile(bw1[None, :], (128, 1)).astype(bf)
    w2p = np.zeros((512, 4), np.float32)
    w2p[0:400] = w2.T
    w2p[511] = bw2
    p["W2s"] = w2p.astype(bf)

    NP = BL * C_
    NPT = (NP + 127) // 128

    in_maps = []
    for c in range(NCORE):
        m = dict(p)
        bs = tokens[c * BL:(c + 1) * BL, 0:T]          # [BL, T]
        tf = np.zeros((CHT * BL, NCH), np.int32)
        tb = np.zeros((CHT * BL, NCH), np.int32)
        for k in range(NCH):
            for tr in range(CHT):
                tf[tr * BL:(tr + 1) * BL, k] = bs[:, k * CHT + tr]
                tb[tr * BL:(tr + 1) * BL, k] = bs[:, T - 1 - (k * CHT + tr)]
        m["tokf"] = tf
        m["tokb"] = tb
        cf = confs[c * BL:(c + 1) * BL]                 # [BL, C, 2]
        t0 = cf[:, :, 0].reshape(-1)
        t1 = cf[:, :, 1].reshape(-1)
        bidx = np.repeat(np.arange(BL), C_)
        ui0 = np.clip(t0, 0, T - 1) * BL + bidx
        ui1 = np.clip(t1, 0, T - 1) * BL + bidx
        um0 = (t0 >= 0).astype(np.float32)
        um1 = (t1 >= 0).astype(np.float32)

        def tile128(a, dt):
            o = np.zeros((NPT * 128,), dt)
            o[:a.shape[0]] = a
            return o.reshape(NPT, 128).T.copy()
        m["uidx0"] = tile128(ui0.astype(np.int32), np.int32)
        m["uidx1"] = tile128(ui1.astype(np.int32), np.int32)
        m["umask0"] = tile128(um0, np.float32)
        m["umask1"] = tile128(um1, np.float32)
        in_maps.append(m)
    return in_maps


_CACHE = {}


def _get_prog(NPT):
    if NPT not in _CACHE:
        _CACHE[NPT] = build(NPT)
    return _CACHE[NPT]


def kernel(**inputs):
    C_ = inputs["confs"].shape[1]
    NP = BL * C_
    NPT = (NP + 127) // 128
    nc = _get_prog(NPT)
    in_maps = prepare_inputs(inputs)
    res = run_bass_kernel_spmd(nc, in_maps, list(range(NCORE)))
    outs = []
    for c in range(NCORE):
        o = res.results[c]["OUT"][:NP]
        outs.append(o)
    return np.concatenate(outs, axis=0).astype(np.float32)


# revision 3
# speedup vs baseline: 1.0000x; 1.0000x over previous
"""Trainium2 Bass kernel for nn_BiLSTMNet — transposed-gates formulation.

Key idea vs baseline: the recurrent/gate matmuls are emitted with the GATE
dimension on PSUM partitions and the (step, lane) batch on the free dim, so
each matmul instruction's cost (= out free size in this cost model) is 16-64
rows instead of 800.  Gate blocks are padded to 256 rows (M=1024, 8 chunks of
128) so sigmoid/tanh and the DVE cell update run as a handful of wide
instructions per step.  h^T is produced directly in matmul-rhs layout (no
per-step transposes), h history is buffered in SBUF for 64 steps and stored
with 2 DMAs per group (HWDGE count ~200 vs ~8000 in the baseline).
The g-gate weights are pre-scaled by 2 so a single sigmoid covers all four
gates (tanh(g) = 2*sigmoid(2g) - 1).
"""
import sys
sys.path.insert(0, "/opt/trn_rl_repo")
import numpy as np
import ml_dtypes

import concourse.bass as bass
import concourse.tile as tile
from concourse import mybir, bacc
from concourse.bass_utils import run_bass_kernel_spmd
from concourse.masks import make_identity

BF16 = mybir.dt.bfloat16
F32 = mybir.dt.float32
I32 = mybir.dt.int32
AF = mybir.ActivationFunctionType
ALU = mybir.AluOpType

V, E, H, B, C = 32000, 200, 200, 128, 256
T = 512
BL = 16            # sentences per core
NCORE = 8
CHT = 4            # timesteps per chunk
NCH = T // CHT     # 128 chunks
GCH = 16           # chunks per h-store group
GSTEP = GCH * CHT  # 64 steps per group
NGRP = NCH // GCH  # 8 groups
NSLOT = T * BL     # 8192
M4 = 1024          # gate-padded M (4 gates x 256)


def build(NPT):
    nc = bacc.Bacc("TRN2", target_bir_lowering=False, debug=False,
                   enable_asserts=True, num_devices=NCORE)

    def din(name, shape, dt):
        return nc.dram_tensor(name, shape, dt, kind="ExternalInput").ap()

    def dout(name, shape, dt):
        return nc.dram_tensor(name, shape, dt, kind="ExternalOutput").ap()

    emb = din("emb", [V, E], BF16)
    W0T = [din(f"W0T{d}", [256, M4], BF16) for d in range(2)]
    Whh0T = [din(f"Whh0T{d}", [200, M4], BF16) for d in range(2)]
    W1T = [din(f"W1T{d}", [401, M4], BF16) for d in range(2)]
    Whh1T = [din(f"Whh1T{d}", [200, M4], BF16) for d in range(2)]
    WUs = din("WUs", [401, 800], BF16)
    W2s = din("W2s", [512, 4], BF16)
    tokf = din("tokf", [CHT * BL, NCH], I32)   # [slot(st,lane), chunk]
    tokb = din("tokb", [CHT * BL, NCH], I32)
    uidx0 = din("uidx0", [128, NPT], I32)
    uidx1 = din("uidx1", [128, NPT], I32)
    umask0 = din("umask0", [128, NPT], F32)
    umask1 = din("umask1", [128, NPT], F32)
    bw1m = din("bw1m", [128, 2 * H], BF16)

    OUT = dout("OUT", [NPT * 128, 4], F32)

    # internal DRAM: h^T in t-major order; rows 0:200 fwd, 200:400 bwd, 400 ones
    h0T = nc.dram_tensor("h0T", [401, NSLOT], BF16).ap()
    h1T = nc.dram_tensor("h1T", [401, NSLOT], BF16).ap()
    U0d = nc.dram_tensor("U0d", [NSLOT, 2 * H], BF16).ap()
    U1d = nc.dram_tensor("U1d", [NSLOT, 2 * H], BF16).ap()

    with tile.TileContext(nc) as tc:
        with tc.tile_pool(name="const", bufs=1) as cp, \
             tc.tile_pool(name="state", bufs=1) as sp:

            def load_tiles(src, rows, ncols, pref):
                tiles = []
                r0 = 0
                for h_ in rows:
                    t_ = cp.tile([h_, ncols], BF16, tag=f"{pref}{r0}",
                                 name=f"{pref}{r0}")
                    nc.sync.dma_start(out=t_[:], in_=src[r0:r0 + h_, :])
                    tiles.append(t_)
                    r0 += h_
                return tiles

            W0t = [load_tiles(W0T[d], [128, 128], M4, f"w0{d}") for d in range(2)]
            Whh0t = [load_tiles(Whh0T[d], [128, 72], M4, f"wh0{d}") for d in range(2)]
            W1t = [load_tiles(W1T[d], [128, 128, 128, 17], M4, f"w1{d}") for d in range(2)]
            Whh1t = [load_tiles(Whh1T[d], [128, 72], M4, f"wh1{d}") for d in range(2)]
            WUt = load_tiles(WUs, [128, 128, 128, 17], 800, "wu")
            W2t = load_tiles(W2s, [128, 128, 128, 128], 4, "w2")

            tok_t = [cp.tile([CHT * BL, NCH], I32, tag=f"tok{d}", name=f"tok{d}")
                     for d in range(2)]
            nc.sync.dma_start(out=tok_t[0][:], in_=tokf[:])
            nc.sync.dma_start(out=tok_t[1][:], in_=tokb[:])

            ones_row = cp.tile([1, NSLOT], BF16, name="ones_row")
            nc.vector.memset(ones_row[:], 1.0)
            nc.sync.dma_start(out=h0T[400:401, :], in_=ones_row[:])
            nc.sync.dma_start(out=h1T[400:401, :], in_=ones_row[:])

            ident64 = sp.tile([64, 64], BF16, name="ident64")
            make_identity(nc, ident64[:])
            ident128 = sp.tile([128, 128], BF16, name="ident128")
            make_identity(nc, ident128[:])

            # persistent LSTM state
            gx = [[sp.tile([CHT * BL, 256], BF16, tag=f"gx{d}{p}", name=f"gx{d}{p}")
                   for p in range(2)] for d in range(2)]
            for d in range(2):
                for p in range(2):
                    nc.vector.memset(gx[d][p][:], 0.0)
                    nc.vector.memset(gx[d][p][:, 255:256], 1.0)
            Hh = [[sp.tile([128, GSTEP * 32], BF16, tag=f"Hh{d}{p}", name=f"Hh{d}{p}")
                   for p in range(2)] for d in range(2)]
            cS = [sp.tile([128, 32], F32, tag=f"cS{d}", name=f"cS{d}")
                  for d in range(2)]
            onesb = sp.tile([128, 32], F32, name="onesb")
            nc.vector.memset(onesb[:], 1.0)

            def hslot(d, p):
                return (p % GSTEP) if d == 0 else (GSTEP - 1 - (p % GSTEP))

            def hpar(p):
                return (p // GSTEP) % 2

            # ============ LSTM layers ============
            with tc.tile_pool(name="work", bufs=2) as wp, \
                 tc.tile_pool(name="pg", bufs=2, space="PSUM") as pgp, \
                 tc.tile_pool(name="xp", bufs=2, space="PSUM") as xpp, \
                 tc.tile_pool(name="rhs1", bufs=2) as rp, \
                 tc.tile_pool(name="uw", bufs=2) as uw, \
                 tc.tile_pool(name="ub", bufs=2) as ubp, \
                 tc.tile_pool(name="ups", bufs=1, space="PSUM") as ups:

                xparts = {}
                pg = {}
                rhs1 = {}

                def emit_x0(k):
                    par = k % 2
                    ps = xpp.tile([128, 256], BF16, space="PSUM",
                                  tag="xps", name="xps",
                                  padded_shape=[128, 1024])
                    for d in range(2):
                        gxt = gx[d][par]
                        nc.gpsimd.indirect_dma_start(
                            out=gxt[:, 0:E], out_offset=None, in_=emb[:],
                            in_offset=bass.IndirectOffsetOnAxis(
                                ap=tok_t[d][:, k:k + 1], axis=0))
                        for half in range(2):
                            c0 = (2 * d + half) * 64
                            nc.tensor.transpose(
                                ps[:, c0:c0 + 64],
                                gxt[:, half * 128:(half + 1) * 128],
                                ident64[:])
                            xt = wp.tile([128, 64], BF16, tag=f"x{d}{half}",
                                         name=f"x{d}{half}")
                            nc.vector.tensor_copy(xt[:], ps[:, c0:c0 + 64])
                            xparts[(k, d, half)] = xt

                def emit_xg0(k):
                    for d in range(2):
                        pgt = pgp.tile([128, 512], F32, space="PSUM",
                                       tag=f"PG{d}", name=f"PG{d}")
                        pg[(k, d)] = pgt
                        for kc in range(2):
                            rhs = xparts.pop((k, d, kc))
                            for mc in range(8):
                                nc.tensor.matmul(
                                    pgt[:, mc * 64:(mc + 1) * 64],
                                    W0t[d][kc][:, mc * 128:(mc + 1) * 128],
                                    rhs[:], start=(kc == 0 and mc == 0),
                                    stop=False, skip_group_check=True)

                def load_rhs1(g):
                    for d in range(2):
                        col0 = g * GSTEP * BL if d == 0 else (T - GSTEP * (g + 1)) * BL
                        tiles = []
                        for (r0, r1) in ((0, 128), (128, 256), (256, 384), (384, 401)):
                            t_ = rp.tile([r1 - r0, GSTEP * BL], BF16,
                                         tag=f"R{d}{r0}", name=f"R{d}{r0}")
                            nc.sync.dma_start(
                                out=t_[:], in_=h0T[r0:r1, col0:col0 + GSTEP * BL])
                            tiles.append(t_)
                        rhs1[(g, d)] = tiles

                def emit_xg1(k):
                    g, cg = k // GCH, k % GCH
                    for d in range(2):
                        pgt = pgp.tile([128, 512], F32, space="PSUM",
                                       tag=f"PG{d}", name=f"PG{d}")
                        pg[(k, d)] = pgt
                        tiles = rhs1[(g, d)]
                        coff = cg * 64 if d == 0 else (GSTEP - 4 - 4 * cg) * BL
                        for kc in range(4):
                            rhs = tiles[kc][:, coff:coff + 64]
                            for mc in range(8):
                                nc.tensor.matmul(
                                    pgt[:, mc * 64:(mc + 1) * 64],
                                    W1t[d][kc][:, mc * 128:(mc + 1) * 128],
                                    rhs, start=(kc == 0 and mc == 0),
                                    stop=False, skip_group_check=True)

                def emit_step_dir(k, st, d, Whht, layer):
                    # full per-direction step sequence; f/b emitted alternately
                    # so the two chains phase-shift on the in-order queues
                    p = CHT * k + st
                    q = st if (d == 0 or layer == 0) else CHT - 1 - st
                    pgt = pg[(k, d)]
                    hs = hslot(d, p - 1)
                    hprev = Hh[d][hpar(p - 1)]
                    for kc in range(2):
                        if kc == 0:
                            rhs = hprev[:, hs * 32:hs * 32 + 16]
                        else:
                            rhs = hprev[0:72, hs * 32 + 16:hs * 32 + 32]
                        for mc in range(8):
                            nc.tensor.matmul(
                                pgt[:, mc * 64 + q * 16:mc * 64 + q * 16 + 16],
                                Whht[d][kc][:, mc * 128:(mc + 1) * 128],
                                rhs, start=False, stop=(kc == 1),
                                skip_group_check=True)
                    G = wp.tile([128, 128], F32, tag=f"G{d}", name=f"G{d}")
                    nc.scalar.activation(
                        G[:].rearrange("p (m s) -> p m s", s=16),
                        pgt[:].rearrange("p (m s) -> p m s", s=64)[:, :, q * 16:(q + 1) * 16],
                        AF.Sigmoid)
                    eng = nc.vector
                    dg = wp.tile([128, 32], F32, tag=f"d{d}", name=f"d{d}")
                    eng.scalar_tensor_tensor(
                        dg[:], G[:, 96:128], 2.0, onesb[:], ALU.mult, ALU.subtract)
                    ag = wp.tile([128, 32], F32, tag=f"a{d}", name=f"a{d}")
                    eng.scalar_tensor_tensor(
                        ag[:], G[:, 32:64], 1.0, dg[:], ALU.mult, ALU.mult)
                    Xg = wp.tile([128, 32], F32, tag=f"X{d}", name=f"X{d}")
                    nc.vector.scalar_tensor_tensor(
                        Xg[:], G[:, 0:32], 1.0, cS[d][:], ALU.mult, ALU.mult)
                    nc.vector.scalar_tensor_tensor(
                        cS[d][:], ag[:], 1.0, Xg[:], ALU.mult, ALU.add)
                    TC = wp.tile([128, 32], F32, tag=f"tc{d}", name=f"tc{d}")
                    nc.scalar.activation(TC[:], cS[d][:], AF.Tanh)
                    hsl = hslot(d, p)
                    nc.vector.scalar_tensor_tensor(
                        Hh[d][hpar(p)][:, hsl * 32:(hsl + 1) * 32],
                        G[:, 64:96], 1.0, TC[:], ALU.mult, ALU.mult)

                def emit_step(k, st, Whht, layer):
                    emit_step_dir(k, st, 0, Whht, layer)
                    emit_step_dir(k, st, 1, Whht, layer)

                def store_h(layer, g):
                    hT = h0T if layer == 0 else h1T
                    par = g % 2
                    for d in range(2):
                        rb = 0 if d == 0 else 200
                        tcol0 = g * GSTEP * BL if d == 0 else (T - GSTEP * (g + 1)) * BL
                        src = Hh[d][par]
                        v = src[:].rearrange("p (s c) -> p s c", c=32)
                        v72 = src[0:72, :].rearrange("p (s c) -> p s c", c=32)
                        nc.sync.dma_start(
                            out=hT[rb:rb + 128, tcol0:tcol0 + GSTEP * BL],
                            in_=v[:, :, 0:16])
                        nc.sync.dma_start(
                            out=hT[rb + 128:rb + 200, tcol0:tcol0 + GSTEP * BL],
                            in_=v72[:, :, 16:32])

                # ---- U phase machinery (interleaved into L1's latency gaps)
                UGC = 8                 # chunks (of 128 slots) per U group
                ustate = {"cur": None, "c8": 0, "lt": None, "Ub": None}
                uready = []

                def u_load_group(ug):
                    c0 = ug * UGC * 128
                    lt = []
                    for (r0, r1) in ((0, 128), (128, 256), (256, 384), (384, 401)):
                        t_ = uw.tile([r1 - r0, UGC * 128], BF16, tag=f"ul{r0}",
                                     name=f"ul{r0}")
                        nc.sync.dma_start(out=t_[:], in_=h1T[r0:r1, c0:c0 + UGC * 128])
                        lt.append(t_)
                    Ub = [ubp.tile([128, UGC * 400], BF16, tag=f"Ub{i}",
                                   name=f"Ub{i}") for i in range(2)]
                    ustate.update(cur=ug, c8=0, lt=lt, Ub=Ub)

                def u_emit_chunk():
                    ug, c8 = ustate["cur"], ustate["c8"]
                    lt, Ub = ustate["lt"], ustate["Ub"]
                    psu = ups.tile([128, 800], F32, space="PSUM", tag="psu",
                                   name="psu")
                    for kc in range(4):
                        for ns in range(7):
                            n0, n1 = ns * 128, min((ns + 1) * 128, 800)
                            nc.tensor.matmul(
                                psu[:, n0:n1],
                                lt[kc][:, c8 * 128:(c8 + 1) * 128],
                                WUt[kc][:, n0:n1],
                                start=(kc == 0 and ns in (0, 4)),
                                stop=(kc == 3), skip_group_check=True)
                    nc.vector.tensor_copy(Ub[0][:, c8 * 400:c8 * 400 + 200],
                                          psu[:, 0:200])
                    nc.vector.tensor_copy(Ub[0][:, c8 * 400 + 200:(c8 + 1) * 400],
                                          psu[:, 200:400])
                    nc.scalar.copy(Ub[1][:, c8 * 400:(c8 + 1) * 400],
                                   psu[:, 400:800])
                    if c8 == UGC - 1:
                        c0 = ug * UGC * 128
                        for i, Ud in enumerate((U0d, U1d)):
                            dst = Ud[c0:c0 + UGC * 128, :].rearrange(
                                "(c p) e -> p c e", p=128)
                            nc.sync.dma_start(
                                out=dst,
                                in_=Ub[i][:].rearrange("p (c e) -> p c e", e=400))
                        ustate["cur"] = None
                    else:
                        ustate["c8"] = c8 + 1

                def u_pump(n):
                    for _ in range(n):
                        if ustate["cur"] is None:
                            if not uready:
                                return
                            u_load_group(uready.pop(0))
                        u_emit_chunk()

                def run_layer(layer):
                    Whht = Whh0t if layer == 0 else Whh1t
                    for d in range(2):
                        nc.vector.memset(cS[d][:], 0.0)
                    # zero the h_prev slots read at p=0 (python-mod indices)
                    nc.vector.memset(
                        Hh[0][hpar(-1)][:, hslot(0, -1) * 32:(hslot(0, -1) + 1) * 32], 0.0)
                    nc.vector.memset(
                        Hh[1][hpar(-1)][:, hslot(1, -1) * 32:(hslot(1, -1) + 1) * 32], 0.0)
                    if layer == 0:
                        emit_x0(0)
                        emit_xg0(0)
                    else:
                        load_rhs1(0)
                        emit_xg1(0)
                    for k in range(NCH):
                        if layer == 1 and k % GCH == 8 and k // GCH + 1 < NGRP:
                            load_rhs1(k // GCH + 1)
                        if layer == 0 and k + 1 < NCH:
                            emit_x0(k + 1)
                        emit_step(k, 0, Whht, layer)
                        emit_step(k, 1, Whht, layer)
                        if k + 1 < NCH:
                            if layer == 0:
                                emit_xg0(k + 1)
                            else:
                                emit_xg1(k + 1)
                        emit_step(k, 2, Whht, layer)
                        emit_step(k, 3, Whht, layer)
                        if layer == 1:
                            u_pump(1)
                        if k % GCH == GCH - 1:
                            g = k // GCH
                            store_h(layer, g)
                            if layer == 1 and g >= NGRP // 2:
                                # U group ug needs L1-f group ug and L1-b group
                                # NGRP-1-ug; both done once g >= max(ug, 7-ug)
                                if g == NGRP - 1:
                                    uready.extend([0, NGRP - 1])
                                else:
                                    uready.extend([NGRP - 1 - g, g])

                run_layer(0)
                run_layer(1)
                # drain remaining U work
                u_pump(NGRP * UGC)

            # ============ gather + MLP ============
            with tc.tile_pool(name="fw", bufs=3) as fw, \
                 tc.tile_pool(name="fc", bufs=1) as fc, \
                 tc.tile_pool(name="ob", bufs=2) as obp, \
                 tc.tile_pool(name="fps", bufs=2, space="PSUM") as fps:
                ui0 = fc.tile([128, NPT], I32, name="ui0")
                ui1 = fc.tile([128, NPT], I32, name="ui1")
                um0 = fc.tile([128, NPT], F32, name="um0")
                um1 = fc.tile([128, NPT], F32, name="um1")
                nc.sync.dma_start(out=ui0[:], in_=uidx0[:])
                nc.sync.dma_start(out=ui1[:], in_=uidx1[:])
                nc.sync.dma_start(out=um0[:], in_=umask0[:])
                nc.sync.dma_start(out=um1[:], in_=umask1[:])
                bwt = fc.tile([128, 2 * H], BF16, name="bwt")
                nc.sync.dma_start(out=bwt[:], in_=bw1m[:])
                hm = [fc.tile([128, 512], BF16, tag=f"hm{i}", name=f"hm{i}")
                      for i in range(3)]
                for t_ in hm:
                    nc.vector.memset(t_[:], 0.0)
                    nc.vector.memset(t_[:, 511:512], 1.0)
                OBW = 8
                for j in range(NPT):
                    par = j % 3
                    if j % OBW == 0:
                        ob = obp.tile([128, OBW * 4], F32, tag="ob", name="ob")
                    g0 = fw.tile([128, 2 * H], BF16, tag="g0", name="g0")
                    g1 = fw.tile([128, 2 * H], BF16, tag="g1", name="g1")
                    nc.gpsimd.indirect_dma_start(
                        out=g0[:], out_offset=None, in_=U0d[:],
                        in_offset=bass.IndirectOffsetOnAxis(ap=ui0[:, j:j + 1], axis=0))
                    nc.gpsimd.indirect_dma_start(
                        out=g1[:], out_offset=None, in_=U1d[:],
                        in_offset=bass.IndirectOffsetOnAxis(ap=ui1[:, j:j + 1], axis=0))
                    g1m = fw.tile([128, 2 * H], BF16, tag="g1m", name="g1m")
                    nc.vector.scalar_tensor_tensor(
                        g1m[:], g1[:], um1[:, j:j + 1], bwt[:], ALU.mult, ALU.add)
                    ssum = fw.tile([128, 2 * H], BF16, tag="ssum", name="ssum")
                    nc.vector.scalar_tensor_tensor(
                        ssum[:], g0[:], um0[:, j:j + 1], g1m[:], ALU.mult, ALU.add)
                    nc.scalar.activation(hm[par][:, 0:2 * H], ssum[:], AF.Tanh)
                    hmT = []
                    pst = fps.tile([128, 512], BF16, space="PSUM",
                                   tag="pst", name="pst",
                                   padded_shape=[128, 1024])
                    for i in range(4):
                        nc.tensor.transpose(
                            pst[:, i * 128:(i + 1) * 128],
                            hm[par][:, i * 128:(i + 1) * 128], ident128[:])
                        ht_ = fw.tile([128, 128], BF16, tag=f"hmT{i}", name=f"hmT{i}")
                        nc.vector.tensor_copy(ht_[:], pst[:, i * 128:(i + 1) * 128])
                        hmT.append(ht_)
                    psl = fps.tile([128, 4], F32, space="PSUM", tag="psl",
                                   name="psl", padded_shape=[128, 512])
                    for i in range(4):
                        nc.tensor.matmul(psl[:], hmT[i][:], W2t[i][:],
                                         start=(i == 0), stop=(i == 3))
                    ex = fw.tile([128, 4], F32, tag="ex", name="ex")
                    nc.scalar.activation(ex[:], psl[:], AF.Exp)
                    sm = fw.tile([128, 1], F32, tag="sm", name="sm")
                    nc.vector.reduce_sum(sm[:], ex[:], axis=mybir.AxisListType.X)
                    rc = fw.tile([128, 1], F32, tag="rc", name="rc")
                    nc.vector.reciprocal(rc[:], sm[:])
                    nc.vector.tensor_scalar_mul(
                        ob[:, (j % OBW) * 4:(j % OBW + 1) * 4], ex[:], rc[:, 0:1])
                    if j % OBW == OBW - 1:
                        j0 = j - OBW + 1
                        dst = OUT[j0 * 128:(j0 + OBW) * 128, :].rearrange(
                            "(a p) e -> p a e", p=128)
                        nc.sync.dma_start(
                            out=dst,
                            in_=ob[:].rearrange("p (a e) -> p a e", e=4))
    nc.compile()
    return nc


# ---------------------------------------------------------------------------
# host-side preparation
# ---------------------------------------------------------------------------

def _perm_scale(w):
    """torch gate order (i,f,g,o) -> (f,i,o,g) along axis 0; scale g block x2."""
    Hq = w.shape[0] // 4
    i, f, g, o = (w[0:Hq], w[Hq:2 * Hq], w[2 * Hq:3 * Hq], w[3 * Hq:4 * Hq])
    return np.concatenate([f, i, o, 2.0 * g], axis=0)


def _gatepad(wT):
    """[K, 800] -> [K, 1024] with each 200-row gate block padded to 256."""
    K = wT.shape[0]
    out = np.zeros((K, M4), np.float32)
    for gb in range(4):
        out[:, gb * 256:gb * 256 + 200] = wT[:, gb * 200:(gb + 1) * 200]
    return out


def prepare_inputs(inputs):
    bf = ml_dtypes.bfloat16
    C_ = np.asarray(inputs["confs"]).shape[1]
    emb = np.asarray(inputs["emb"], np.float32)
    tokens = np.asarray(inputs["tokens"])
    confs = np.asarray(inputs["confs"])

    p = {}
    p["emb"] = emb.astype(bf)

    def prep_dir(Wih, Whh, b, kin, kpad, name_w, name_h):
        Wp = _perm_scale(np.asarray(Wih, np.float32))      # [800, kin]
        bp = _perm_scale(np.asarray(b, np.float32))        # [800]
        Hp = _perm_scale(np.asarray(Whh, np.float32))      # [800, 200]
        wt = np.zeros((kpad, M4), np.float32)
        wt[0:kin] = _gatepad(Wp.T)
        wt[kpad - 1] = _gatepad(bp[None, :])[0]
        p[name_w] = wt.astype(bf)
        p[name_h] = _gatepad(Hp.T).astype(bf)

    prep_dir(inputs["Wih0f"], inputs["Whh0f"], inputs["b0f"], E, 256, "W0T0", "Whh0T0")
    prep_dir(inputs["Wih0b"], inputs["Whh0b"], inputs["b0b"], E, 256, "W0T1", "Whh0T1")
    prep_dir(inputs["Wih1f"], inputs["Whh1f"], inputs["b1f"], 400, 401, "W1T0", "Whh1T0")
    prep_dir(inputs["Wih1b"], inputs["Whh1b"], inputs["b1b"], 400, 401, "W1T1", "Whh1T1")

    w1 = np.asarray(inputs["w1"], np.float32)
    bw1 = np.asarray(inputs["bw1"], np.float32)
    w2 = np.asarray(inputs["w2"], np.float32)
    bw2 = np.asarray(inputs["bw2"], np.float32)

    wu = np.zeros((401, 800), np.float32)
    wu[0:400, 0:400] = w1[:, 0:400].T
    wu[0:400, 400:800] = w1[:, 400:800].T
    p["WUs"] = wu.astype(bf)
    p["bw1m"] = np.tile(bw1[None, :], (128, 1)).astype(bf)
    w2p = np.zeros((512, 4), np.float32)
    w2p[0:400] = w2.T
    w2p[511] = bw2
    p["W2s"] = w2p.astype(bf)

    NP = BL * C_
    NPT = (NP + 127) // 128

    in_maps = []
    for c in range(NCORE):
        m = dict(p)
        bs = tokens[c * BL:(c + 1) * BL, 0:T]          # [BL, T]
        tf = np.zeros((CHT * BL, NCH), np.int32)
        tb = np.zeros((CHT * BL, NCH), np.int32)
        for k in range(NCH):
            for tr in range(CHT):
                tf[tr * BL:(tr + 1) * BL, k] = bs[:, k * CHT + tr]
                tb[tr * BL:(tr + 1) * BL, k] = bs[:, T - 1 - (k * CHT + tr)]
        m["tokf"] = tf
        m["tokb"] = tb
        cf = confs[c * BL:(c + 1) * BL]                 # [BL, C, 2]
        t0 = cf[:, :, 0].reshape(-1)
        t1 = cf[:, :, 1].reshape(-1)
        bidx = np.repeat(np.arange(BL), C_)
        ui0 = np.clip(t0, 0, T - 1) * BL + bidx
        ui1 = np.clip(t1, 0, T - 1) * BL + bidx
        um0 = (t0 >= 0).astype(np.float32)
        um1 = (t1 >= 0).astype(np.float32)

        def tile128(a, dt):
            o = np.zeros((NPT * 128,), dt)
            o[:a.shape[0]] = a
            return o.reshape(NPT, 128).T.copy()
        m["uidx0"] = tile128(ui0.astype(np.int32), np.int32)
        m["uidx1"] = tile128(ui1.astype(np.int32), np.int32)
        m["umask0"] = tile128(um0, np.float32)
        m["umask1"] = tile128(um1, np.float32)
        in_maps.append(m)
    return in_maps


_CACHE = {}


def _get_prog(NPT):
    if NPT not in _CACHE:
        _CACHE[NPT] = build(NPT)
    return _CACHE[NPT]


def kernel(**inputs):
    C_ = inputs["confs"].shape[1]
    NP = BL * C_
    NPT = (NP + 127) // 128
    nc = _get_prog(NPT)
    in_maps = prepare_inputs(inputs)
    res = run_bass_kernel_spmd(nc, in_maps, list(range(NCORE)))
    outs = []
    for c in range(NCORE):
        o = res.results[c]["OUT"][:NP]
        outs.append(o)
    return np.concatenate(outs, axis=0).astype(np.float32)


# revision 4
# speedup vs baseline: 1.0065x; 1.0065x over previous
"""Trainium2 Bass kernel for nn_BiLSTMNet — transposed-gates formulation.

Key idea vs baseline: the recurrent/gate matmuls are emitted with the GATE
dimension on PSUM partitions and the (step, lane) batch on the free dim, so
each matmul instruction's cost (= out free size in this cost model) is 16-64
rows instead of 800.  Gate blocks are padded to 256 rows (M=1024, 8 chunks of
128) so sigmoid/tanh and the DVE cell update run as a handful of wide
instructions per step.  h^T is produced directly in matmul-rhs layout (no
per-step transposes), h history is buffered in SBUF for 64 steps and stored
with 2 DMAs per group (HWDGE count ~200 vs ~8000 in the baseline).
The g-gate weights are pre-scaled by 2 so a single sigmoid covers all four
gates (tanh(g) = 2*sigmoid(2g) - 1).
"""
import sys
sys.path.insert(0, "/opt/trn_rl_repo")
import numpy as np
import ml_dtypes

import concourse.bass as bass
import concourse.tile as tile
from concourse import mybir, bacc
from concourse.bass_utils import run_bass_kernel_spmd
from concourse.masks import make_identity

BF16 = mybir.dt.bfloat16
F32 = mybir.dt.float32
I32 = mybir.dt.int32
AF = mybir.ActivationFunctionType
ALU = mybir.AluOpType

V, E, H, B, C = 32000, 200, 200, 128, 256
T = 512
BL = 16            # sentences per core
NCORE = 8
CHT = 4            # timesteps per chunk
NCH = T // CHT     # 128 chunks
GCH = 16           # chunks per h-store group
GSTEP = GCH * CHT  # 64 steps per group
NGRP = NCH // GCH  # 8 groups
NSLOT = T * BL     # 8192
M4 = 1024          # gate-padded M (4 gates x 256)


def build(NPT):
    nc = bacc.Bacc("TRN2", target_bir_lowering=False, debug=False,
                   enable_asserts=True, num_devices=NCORE)

    def din(name, shape, dt):
        return nc.dram_tensor(name, shape, dt, kind="ExternalInput").ap()

    def dout(name, shape, dt):
        return nc.dram_tensor(name, shape, dt, kind="ExternalOutput").ap()

    emb = din("emb", [V, E], BF16)
    W0T = [din(f"W0T{d}", [256, M4], BF16) for d in range(2)]
    Whh0T = [din(f"Whh0T{d}", [200, M4], BF16) for d in range(2)]
    W1T = [din(f"W1T{d}", [401, M4], BF16) for d in range(2)]
    Whh1T = [din(f"Whh1T{d}", [200, M4], BF16) for d in range(2)]
    WUs = din("WUs", [401, 800], BF16)
    W2s = din("W2s", [512, 4], BF16)
    tokf = din("tokf", [CHT * BL, NCH], I32)   # [slot(st,lane), chunk]
    tokb = din("tokb", [CHT * BL, NCH], I32)
    uidx0 = din("uidx0", [128, NPT], I32)
    uidx1 = din("uidx1", [128, NPT], I32)
    umask0 = din("umask0", [128, NPT], F32)
    umask1 = din("umask1", [128, NPT], F32)
    bw1m = din("bw1m", [128, 2 * H], BF16)

    OUT = dout("OUT", [NPT * 128, 4], F32)

    # internal DRAM: h^T in t-major order; rows 0:200 fwd, 200:400 bwd, 400 ones
    h0T = nc.dram_tensor("h0T", [401, NSLOT], BF16).ap()
    h1T = nc.dram_tensor("h1T", [401, NSLOT], BF16).ap()
    U0d = nc.dram_tensor("U0d", [NSLOT, 2 * H], BF16).ap()
    U1d = nc.dram_tensor("U1d", [NSLOT, 2 * H], BF16).ap()

    with tile.TileContext(nc) as tc:
        with tc.tile_pool(name="const", bufs=1) as cp, \
             tc.tile_pool(name="state", bufs=1) as sp:

            def load_tiles(src, rows, ncols, pref):
                tiles = []
                r0 = 0
                for h_ in rows:
                    t_ = cp.tile([h_, ncols], BF16, tag=f"{pref}{r0}",
                                 name=f"{pref}{r0}")
                    nc.sync.dma_start(out=t_[:], in_=src[r0:r0 + h_, :])
                    tiles.append(t_)
                    r0 += h_
                return tiles

            W0t = [load_tiles(W0T[d], [128, 128], M4, f"w0{d}") for d in range(2)]
            Whh0t = [load_tiles(Whh0T[d], [128, 72], M4, f"wh0{d}") for d in range(2)]
            W1t = [load_tiles(W1T[d], [128, 128, 128, 17], M4, f"w1{d}") for d in range(2)]
            Whh1t = [load_tiles(Whh1T[d], [128, 72], M4, f"wh1{d}") for d in range(2)]
            WUt = load_tiles(WUs, [128, 128, 128, 17], 800, "wu")
            W2t = load_tiles(W2s, [128, 128, 128, 128], 4, "w2")

            tok_t = [cp.tile([CHT * BL, NCH], I32, tag=f"tok{d}", name=f"tok{d}")
                     for d in range(2)]
            nc.sync.dma_start(out=tok_t[0][:], in_=tokf[:])
            nc.sync.dma_start(out=tok_t[1][:], in_=tokb[:])

            ones_row = cp.tile([1, NSLOT], BF16, name="ones_row")
            nc.vector.memset(ones_row[:], 1.0)
            nc.sync.dma_start(out=h0T[400:401, :], in_=ones_row[:])
            nc.sync.dma_start(out=h1T[400:401, :], in_=ones_row[:])

            ident64 = sp.tile([64, 64], BF16, name="ident64")
            make_identity(nc, ident64[:])
            ident128 = sp.tile([128, 128], BF16, name="ident128")
            make_identity(nc, ident128[:])

            # persistent LSTM state
            gx = [[sp.tile([CHT * BL, 256], BF16, tag=f"gx{d}{p}", name=f"gx{d}{p}")
                   for p in range(2)] for d in range(2)]
            for d in range(2):
                for p in range(2):
                    nc.vector.memset(gx[d][p][:], 0.0)
                    nc.vector.memset(gx[d][p][:, 255:256], 1.0)
            Hh = [[sp.tile([128, GSTEP * 32], BF16, tag=f"Hh{d}{p}", name=f"Hh{d}{p}")
                   for p in range(2)] for d in range(2)]
            cS = [sp.tile([128, 32], F32, tag=f"cS{d}", name=f"cS{d}")
                  for d in range(2)]
            onesb = sp.tile([128, 32], F32, name="onesb")
            nc.vector.memset(onesb[:], 1.0)

            def hslot(d, p):
                return (p % GSTEP) if d == 0 else (GSTEP - 1 - (p % GSTEP))

            def hpar(p):
                return (p // GSTEP) % 2

            # ============ LSTM layers ============
            with tc.tile_pool(name="work", bufs=3) as wp, \
                 tc.tile_pool(name="pg", bufs=2, space="PSUM") as pgp, \
                 tc.tile_pool(name="xp", bufs=2, space="PSUM") as xpp, \
                 tc.tile_pool(name="rhs1", bufs=2) as rp, \
                 tc.tile_pool(name="uw", bufs=2) as uw, \
                 tc.tile_pool(name="ub", bufs=2) as ubp, \
                 tc.tile_pool(name="ups", bufs=1, space="PSUM") as ups:

                xparts = {}
                pg = {}
                rhs1 = {}

                def emit_x0(k):
                    par = k % 2
                    ps = xpp.tile([128, 256], BF16, space="PSUM",
                                  tag="xps", name="xps",
                                  padded_shape=[128, 1024])
                    for d in range(2):
                        gxt = gx[d][par]
                        nc.gpsimd.indirect_dma_start(
                            out=gxt[:, 0:E], out_offset=None, in_=emb[:],
                            in_offset=bass.IndirectOffsetOnAxis(
                                ap=tok_t[d][:, k:k + 1], axis=0))
                        for half in range(2):
                            c0 = (2 * d + half) * 64
                            nc.tensor.transpose(
                                ps[:, c0:c0 + 64],
                                gxt[:, half * 128:(half + 1) * 128],
                                ident64[:])
                            xt = wp.tile([128, 64], BF16, tag=f"x{d}{half}",
                                         name=f"x{d}{half}")
                            nc.vector.tensor_copy(xt[:], ps[:, c0:c0 + 64])
                            xparts[(k, d, half)] = xt

                def emit_xg0(k):
                    for d in range(2):
                        pgt = pgp.tile([128, 512], F32, space="PSUM",
                                       tag=f"PG{d}", name=f"PG{d}")
                        pg[(k, d)] = pgt
                        for kc in range(2):
                            rhs = xparts.pop((k, d, kc))
                            for mc in range(8):
                                nc.tensor.matmul(
                                    pgt[:, mc * 64:(mc + 1) * 64],
                                    W0t[d][kc][:, mc * 128:(mc + 1) * 128],
                                    rhs[:], start=(kc == 0 and mc == 0),
                                    stop=False, skip_group_check=True)

                def load_rhs1(g):
                    for d in range(2):
                        col0 = g * GSTEP * BL if d == 0 else (T - GSTEP * (g + 1)) * BL
                        tiles = []
                        for (r0, r1) in ((0, 128), (128, 256), (256, 384), (384, 401)):
                            t_ = rp.tile([r1 - r0, GSTEP * BL], BF16,
                                         tag=f"R{d}{r0}", name=f"R{d}{r0}")
                            nc.sync.dma_start(
                                out=t_[:], in_=h0T[r0:r1, col0:col0 + GSTEP * BL])
                            tiles.append(t_)
                        rhs1[(g, d)] = tiles

                def emit_xg1(k):
                    g, cg = k // GCH, k % GCH
                    for d in range(2):
                        pgt = pgp.tile([128, 512], F32, space="PSUM",
                                       tag=f"PG{d}", name=f"PG{d}")
                        pg[(k, d)] = pgt
                        tiles = rhs1[(g, d)]
                        coff = cg * 64 if d == 0 else (GSTEP - 4 - 4 * cg) * BL
                        for kc in range(4):
                            rhs = tiles[kc][:, coff:coff + 64]
                            for mc in range(8):
                                nc.tensor.matmul(
                                    pgt[:, mc * 64:(mc + 1) * 64],
                                    W1t[d][kc][:, mc * 128:(mc + 1) * 128],
                                    rhs, start=(kc == 0 and mc == 0),
                                    stop=False, skip_group_check=True)

                def emit_step_dir(k, st, d, Whht, layer):
                    # full per-direction step sequence; f/b emitted alternately
                    # so the two chains phase-shift on the in-order queues
                    p = CHT * k + st
                    q = st if (d == 0 or layer == 0) else CHT - 1 - st
                    pgt = pg[(k, d)]
                    hs = hslot(d, p - 1)
                    hprev = Hh[d][hpar(p - 1)]
                    for kc in range(2):
                        if kc == 0:
                            rhs = hprev[:, hs * 32:hs * 32 + 16]
                        else:
                            rhs = hprev[0:72, hs * 32 + 16:hs * 32 + 32]
                        for mc in range(8):
                            nc.tensor.matmul(
                                pgt[:, mc * 64 + q * 16:mc * 64 + q * 16 + 16],
                                Whht[d][kc][:, mc * 128:(mc + 1) * 128],
                                rhs, start=False, stop=(kc == 1),
                                skip_group_check=True)
                    G = wp.tile([128, 128], F32, tag=f"G{d}", name=f"G{d}")
                    nc.scalar.activation(
                        G[:].rearrange("p (m s) -> p m s", s=16),
                        pgt[:].rearrange("p (m s) -> p m s", s=64)[:, :, q * 16:(q + 1) * 16],
                        AF.Sigmoid)
                    eng = nc.vector
                    dg = wp.tile([128, 32], F32, tag=f"d{d}", name=f"d{d}")
                    eng.scalar_tensor_tensor(
                        dg[:], G[:, 96:128], 2.0, onesb[:], ALU.mult, ALU.subtract)
                    ag = wp.tile([128, 32], F32, tag=f"a{d}", name=f"a{d}")
                    eng.scalar_tensor_tensor(
                        ag[:], G[:, 32:64], 1.0, dg[:], ALU.mult, ALU.mult)
                    Xg = wp.tile([128, 32], F32, tag=f"X{d}", name=f"X{d}")
                    nc.vector.scalar_tensor_tensor(
                        Xg[:], G[:, 0:32], 1.0, cS[d][:], ALU.mult, ALU.mult)
                    nc.vector.scalar_tensor_tensor(
                        cS[d][:], ag[:], 1.0, Xg[:], ALU.mult, ALU.add)
                    TC = wp.tile([128, 32], F32, tag=f"tc{d}", name=f"tc{d}")
                    nc.scalar.activation(TC[:], cS[d][:], AF.Tanh)
                    hsl = hslot(d, p)
                    nc.vector.scalar_tensor_tensor(
                        Hh[d][hpar(p)][:, hsl * 32:(hsl + 1) * 32],
                        G[:, 64:96], 1.0, TC[:], ALU.mult, ALU.mult)

                def emit_step(k, st, Whht, layer):
                    emit_step_dir(k, st, 0, Whht, layer)
                    emit_step_dir(k, st, 1, Whht, layer)

                def store_h(layer, g):
                    hT = h0T if layer == 0 else h1T
                    par = g % 2
                    for d in range(2):
                        rb = 0 if d == 0 else 200
                        tcol0 = g * GSTEP * BL if d == 0 else (T - GSTEP * (g + 1)) * BL
                        src = Hh[d][par]
                        v = src[:].rearrange("p (s c) -> p s c", c=32)
                        v72 = src[0:72, :].rearrange("p (s c) -> p s c", c=32)
                        nc.sync.dma_start(
                            out=hT[rb:rb + 128, tcol0:tcol0 + GSTEP * BL],
                            in_=v[:, :, 0:16])
                        nc.sync.dma_start(
                            out=hT[rb + 128:rb + 200, tcol0:tcol0 + GSTEP * BL],
                            in_=v72[:, :, 16:32])

                # ---- U phase machinery (interleaved into L1's latency gaps)
                UGC = 8                 # chunks (of 128 slots) per U group
                ustate = {"cur": None, "c8": 0, "lt": None, "Ub": None}
                uready = []

                def u_load_group(ug):
                    c0 = ug * UGC * 128
                    lt = []
                    for (r0, r1) in ((0, 128), (128, 256), (256, 384), (384, 401)):
                        t_ = uw.tile([r1 - r0, UGC * 128], BF16, tag=f"ul{r0}",
                                     name=f"ul{r0}")
                        nc.sync.dma_start(out=t_[:], in_=h1T[r0:r1, c0:c0 + UGC * 128])
                        lt.append(t_)
                    Ub = [ubp.tile([128, UGC * 400], BF16, tag=f"Ub{i}",
                                   name=f"Ub{i}") for i in range(2)]
                    ustate.update(cur=ug, c8=0, lt=lt, Ub=Ub)

                def u_emit_chunk():
                    ug, c8 = ustate["cur"], ustate["c8"]
                    lt, Ub = ustate["lt"], ustate["Ub"]
                    psu = ups.tile([128, 800], F32, space="PSUM", tag="psu",
                                   name="psu")
                    for kc in range(4):
                        for ns in range(7):
                            n0, n1 = ns * 128, min((ns + 1) * 128, 800)
                            nc.tensor.matmul(
                                psu[:, n0:n1],
                                lt[kc][:, c8 * 128:(c8 + 1) * 128],
                                WUt[kc][:, n0:n1],
                                start=(kc == 0 and ns in (0, 4)),
                                stop=(kc == 3), skip_group_check=True)
                    nc.vector.tensor_copy(Ub[0][:, c8 * 400:c8 * 400 + 200],
                                          psu[:, 0:200])
                    nc.vector.tensor_copy(Ub[0][:, c8 * 400 + 200:(c8 + 1) * 400],
                                          psu[:, 200:400])
                    nc.scalar.copy(Ub[1][:, c8 * 400:(c8 + 1) * 400],
                                   psu[:, 400:800])
                    if c8 == UGC - 1:
                        c0 = ug * UGC * 128
                        for i, Ud in enumerate((U0d, U1d)):
                            dst = Ud[c0:c0 + UGC * 128, :].rearrange(
                                "(c p) e -> p c e", p=128)
                            nc.sync.dma_start(
                                out=dst,
                                in_=Ub[i][:].rearrange("p (c e) -> p c e", e=400))
                        ustate["cur"] = None
                    else:
                        ustate["c8"] = c8 + 1

                def u_pump(n):
                    for _ in range(n):
                        if ustate["cur"] is None:
                            if not uready:
                                return
                            u_load_group(uready.pop(0))
                        u_emit_chunk()

                def run_layer(layer):
                    Whht = Whh0t if layer == 0 else Whh1t
                    for d in range(2):
                        nc.vector.memset(cS[d][:], 0.0)
                    # zero the h_prev slots read at p=0 (python-mod indices)
                    nc.vector.memset(
                        Hh[0][hpar(-1)][:, hslot(0, -1) * 32:(hslot(0, -1) + 1) * 32], 0.0)
                    nc.vector.memset(
                        Hh[1][hpar(-1)][:, hslot(1, -1) * 32:(hslot(1, -1) + 1) * 32], 0.0)
                    if layer == 0:
                        emit_x0(0)
                        emit_xg0(0)
                    else:
                        load_rhs1(0)
                        emit_xg1(0)
                    for k in range(NCH):
                        if layer == 1 and k % GCH == 8 and k // GCH + 1 < NGRP:
                            load_rhs1(k // GCH + 1)
                        if layer == 0 and k + 1 < NCH:
                            emit_x0(k + 1)
                        emit_step(k, 0, Whht, layer)
                        emit_step(k, 1, Whht, layer)
                        if k + 1 < NCH:
                            if layer == 0:
                                emit_xg0(k + 1)
                            else:
                                emit_xg1(k + 1)
                        emit_step(k, 2, Whht, layer)
                        emit_step(k, 3, Whht, layer)
                        if layer == 1:
                            u_pump(1)
                        if k % GCH == GCH - 1:
                            g = k // GCH
                            store_h(layer, g)
                            if layer == 1 and g >= NGRP // 2:
                                # U group ug needs L1-f group ug and L1-b group
                                # NGRP-1-ug; both done once g >= max(ug, 7-ug)
                                if g == NGRP - 1:
                                    uready.extend([0, NGRP - 1])
                                else:
                                    uready.extend([NGRP - 1 - g, g])

                run_layer(0)
                run_layer(1)
                # drain remaining U work
                u_pump(NGRP * UGC)

            # ============ gather + MLP ============
            with tc.tile_pool(name="fw", bufs=4) as fw, \
                 tc.tile_pool(name="fc", bufs=1) as fc, \
                 tc.tile_pool(name="ob", bufs=2) as obp, \
                 tc.tile_pool(name="fps", bufs=2, space="PSUM") as fps:
                ui0 = fc.tile([128, NPT], I32, name="ui0")
                ui1 = fc.tile([128, NPT], I32, name="ui1")
                um0 = fc.tile([128, NPT], F32, name="um0")
                um1 = fc.tile([128, NPT], F32, name="um1")
                nc.sync.dma_start(out=ui0[:], in_=uidx0[:])
                nc.sync.dma_start(out=ui1[:], in_=uidx1[:])
                nc.sync.dma_start(out=um0[:], in_=umask0[:])
                nc.sync.dma_start(out=um1[:], in_=umask1[:])
                bwt = fc.tile([128, 2 * H], BF16, name="bwt")
                nc.sync.dma_start(out=bwt[:], in_=bw1m[:])
                hm = [fc.tile([128, 512], BF16, tag=f"hm{i}", name=f"hm{i}")
                      for i in range(3)]
                for t_ in hm:
                    nc.vector.memset(t_[:], 0.0)
                    nc.vector.memset(t_[:, 511:512], 1.0)
                OBW = 8
                for j in range(NPT):
                    par = j % 3
                    if j % OBW == 0:
                        ob = obp.tile([128, OBW * 4], F32, tag="ob", name="ob")
                    g0 = fw.tile([128, 2 * H], BF16, tag="g0", name="g0")
                    g1 = fw.tile([128, 2 * H], BF16, tag="g1", name="g1")
                    nc.gpsimd.indirect_dma_start(
                        out=g0[:], out_offset=None, in_=U0d[:],
                        in_offset=bass.IndirectOffsetOnAxis(ap=ui0[:, j:j + 1], axis=0))
                    nc.gpsimd.indirect_dma_start(
                        out=g1[:], out_offset=None, in_=U1d[:],
                        in_offset=bass.IndirectOffsetOnAxis(ap=ui1[:, j:j + 1], axis=0))
                    g1m = fw.tile([128, 2 * H], BF16, tag="g1m", name="g1m")
                    nc.vector.scalar_tensor_tensor(
                        g1m[:], g1[:], um1[:, j:j + 1], bwt[:], ALU.mult, ALU.add)
                    ssum = fw.tile([128, 2 * H], BF16, tag="ssum", name="ssum")
                    nc.vector.scalar_tensor_tensor(
                        ssum[:], g0[:], um0[:, j:j + 1], g1m[:], ALU.mult, ALU.add)
                    nc.scalar.activation(hm[par][:, 0:2 * H], ssum[:], AF.Tanh)
                    hmT = []
                    pst = fps.tile([128, 512], BF16, space="PSUM",
                                   tag="pst", name="pst",
                                   padded_shape=[128, 1024])
                    for i in range(4):
                        nc.tensor.transpose(
                            pst[:, i * 128:(i + 1) * 128],
                            hm[par][:, i * 128:(i + 1) * 128], ident128[:])
                        ht_ = fw.tile([128, 128], BF16, tag=f"hmT{i}", name=f"hmT{i}")
                        nc.vector.tensor_copy(ht_[:], pst[:, i * 128:(i + 1) * 128])
                        hmT.append(ht_)
                    psl = fps.tile([128, 4], F32, space="PSUM", tag="psl",
                                   name="psl", padded_shape=[128, 512])
                    for i in range(4):
                        nc.tensor.matmul(psl[:], hmT[i][:], W2t[i][:],
                                         start=(i == 0), stop=(i == 3))
                    ex = fw.tile([128, 4], F32, tag="ex", name="ex")
                    nc.scalar.activation(ex[:], psl[:], AF.Exp)
                    sm = fw.tile([128, 1], F32, tag="sm", name="sm")
                    nc.vector.reduce_sum(sm[:], ex[:], axis=mybir.AxisListType.X)
                    rc = fw.tile([128, 1], F32, tag="rc", name="rc")
                    nc.vector.reciprocal(rc[:], sm[:])
                    nc.vector.tensor_scalar_mul(
                        ob[:, (j % OBW) * 4:(j % OBW + 1) * 4], ex[:], rc[:, 0:1])
                    if j % OBW == OBW - 1:
                        j0 = j - OBW + 1
                        dst = OUT[j0 * 128:(j0 + OBW) * 128, :].rearrange(
                            "(a p) e -> p a e", p=128)
                        nc.sync.dma_start(
                            out=dst,
                            in_=ob[:].rearrange("p (a e) -> p a e", e=4))
    nc.compile()
    return nc


# ---------------------------------------------------------------------------
# host-side preparation
# ---------------------------------------------------------------------------

def _perm_scale(w):
    """torch gate order (i,f,g,o) -> (f,i,o,g) along axis 0; scale g block x2."""
    Hq = w.shape[0] // 4
    i, f, g, o = (w[0:Hq], w[Hq:2 * Hq], w[2 * Hq:3 * Hq], w[3 * Hq:4 * Hq])
    return np.concatenate([f, i, o, 2.0 * g], axis=0)


def _gatepad(wT):
    """[K, 800] -> [K, 1024] with each 200-row gate block padded to 256."""
    K = wT.shape[0]
    out = np.zeros((K, M4), np.float32)
    for gb in range(4):
        out[:, gb * 256:gb * 256 + 200] = wT[:, gb * 200:(gb + 1) * 200]
    return out


def prepare_inputs(inputs):
    bf = ml_dtypes.bfloat16
    C_ = np.asarray(inputs["confs"]).shape[1]
    emb = np.asarray(inputs["emb"], np.float32)
    tokens = np.asarray(inputs["tokens"])
    confs = np.asarray(inputs["confs"])

    p = {}
    p["emb"] = emb.astype(bf)

    def prep_dir(Wih, Whh, b, kin, kpad, name_w, name_h):
        Wp = _perm_scale(np.asarray(Wih, np.float32))      # [800, kin]
        bp = _perm_scale(np.asarray(b, np.float32))        # [800]
        Hp = _perm_scale(np.asarray(Whh, np.float32))      # [800, 200]
        wt = np.zeros((kpad, M4), np.float32)
        wt[0:kin] = _gatepad(Wp.T)
        wt[kpad - 1] = _gatepad(bp[None, :])[0]
        p[name_w] = wt.astype(bf)
        p[name_h] = _gatepad(Hp.T).astype(bf)

    prep_dir(inputs["Wih0f"], inputs["Whh0f"], inputs["b0f"], E, 256, "W0T0", "Whh0T0")
    prep_dir(inputs["Wih0b"], inputs["Whh0b"], inputs["b0b"], E, 256, "W0T1", "Whh0T1")
    prep_dir(inputs["Wih1f"], inputs["Whh1f"], inputs["b1f"], 400, 401, "W1T0", "Whh1T0")
    prep_dir(inputs["Wih1b"], inputs["Whh1b"], inputs["b1b"], 400, 401, "W1T1", "Whh1T1")

    w1 = np.asarray(inputs["w1"], np.float32)
    bw1 = np.asarray(inputs["bw1"], np.float32)
    w2 = np.asarray(inputs["w2"], np.float32)
    bw2 = np.asarray(inputs["bw2"], np.float32)

    wu = np.zeros((401, 800), np.float32)
    wu[0:400, 0:400] = w1[:, 0:400].T
    wu[0:400, 400:800] = w1[:, 400:800].T
    p["WUs"] = wu.astype(bf)
    p["bw1m"] = np.tile(bw1[None, :], (128, 1)).astype(bf)
    w2p = np.zeros((512, 4), np.float32)
    w2p[0:400] = w2.T
    w2p[511] = bw2
    p["W2s"] = w2p.astype(bf)

    NP = BL * C_
    NPT = (NP + 127) // 128

    in_maps = []
    for c in range(NCORE):
        m = dict(p)
        bs = tokens[c * BL:(c + 1) * BL, 0:T]          # [BL, T]
        tf = np.zeros((CHT * BL, NCH), np.int32)
        tb = np.zeros((CHT * BL, NCH), np.int32)
        for k in range(NCH):
            for tr in range(CHT):
                tf[tr * BL:(tr + 1) * BL, k] = bs[:, k * CHT + tr]
                tb[tr * BL:(tr + 1) * BL, k] = bs[:, T - 1 - (k * CHT + tr)]
        m["tokf"] = tf
        m["tokb"] = tb
        cf = confs[c * BL:(c + 1) * BL]                 # [BL, C, 2]
        t0 = cf[:, :, 0].reshape(-1)
        t1 = cf[:, :, 1].reshape(-1)
        bidx = np.repeat(np.arange(BL), C_)
        ui0 = np.clip(t0, 0, T - 1) * BL + bidx
        ui1 = np.clip(t1, 0, T - 1) * BL + bidx
        um0 = (t0 >= 0).astype(np.float32)
        um1 = (t1 >= 0).astype(np.float32)

        def tile128(a, dt):
            o = np.zeros((NPT * 128,), dt)
            o[:a.shape[0]] = a
            return o.reshape(NPT, 128).T.copy()
        m["uidx0"] = tile128(ui0.astype(np.int32), np.int32)
        m["uidx1"] = tile128(ui1.astype(np.int32), np.int32)
        m["umask0"] = tile128(um0, np.float32)
        m["umask1"] = tile128(um1, np.float32)
        in_maps.append(m)
    return in_maps


_CACHE = {}


def _get_prog(NPT):
    if NPT not in _CACHE:
        _CACHE[NPT] = build(NPT)
    return _CACHE[NPT]


def kernel(**inputs):
    C_ = inputs["confs"].shape[1]
    NP = BL * C_
    NPT = (NP + 127) // 128
    nc = _get_prog(NPT)
    in_maps = prepare_inputs(inputs)
    res = run_bass_kernel_spmd(nc, in_maps, list(range(NCORE)))
    outs = []
    for c in range(NCORE):
        o = res.results[c]["OUT"][:NP]
        outs.append(o)
    return np.concatenate(outs, axis=0).astype(np.float32)


# revision 5
# speedup vs baseline: 1.0065x; 1.0000x over previous
"""Trainium2 Bass kernel for nn_BiLSTMNet — transposed-gates formulation.

Key idea vs baseline: the recurrent/gate matmuls are emitted with the GATE
dimension on PSUM partitions and the (step, lane) batch on the free dim, so
each matmul instruction's cost (= out free size in this cost model) is 16-64
rows instead of 800.  Gate blocks are padded to 256 rows (M=1024, 8 chunks of
128) so sigmoid/tanh and the DVE cell update run as a handful of wide
instructions per step.  h^T is produced directly in matmul-rhs layout (no
per-step transposes), h history is buffered in SBUF for 64 steps and stored
with 2 DMAs per group (HWDGE count ~200 vs ~8000 in the baseline).
The g-gate weights are pre-scaled by 2 so a single sigmoid covers all four
gates (tanh(g) = 2*sigmoid(2g) - 1).
"""
import sys
sys.path.insert(0, "/opt/trn_rl_repo")
import numpy as np
import ml_dtypes

import concourse.bass as bass
import concourse.tile as tile
from concourse import mybir, bacc
from concourse.bass_utils import run_bass_kernel_spmd
from concourse.masks import make_identity

BF16 = mybir.dt.bfloat16
F32 = mybir.dt.float32
I32 = mybir.dt.int32
AF = mybir.ActivationFunctionType
ALU = mybir.AluOpType

V, E, H, B, C = 32000, 200, 200, 128, 256
T = 512
BL = 16            # sentences per core
NCORE = 8
CHT = 4            # timesteps per chunk
NCH = T // CHT     # 128 chunks
GCH = 16           # chunks per h-store group
GSTEP = GCH * CHT  # 64 steps per group
NGRP = NCH // GCH  # 8 groups
NSLOT = T * BL     # 8192
M4 = 1024          # gate-padded M (4 gates x 256)


def build(NPT):
    nc = bacc.Bacc("TRN2", target_bir_lowering=False, debug=False,
                   enable_asserts=True, num_devices=NCORE)

    def din(name, shape, dt):
        return nc.dram_tensor(name, shape, dt, kind="ExternalInput").ap()

    def dout(name, shape, dt):
        return nc.dram_tensor(name, shape, dt, kind="ExternalOutput").ap()

    emb = din("emb", [V, E], BF16)
    W0T = [din(f"W0T{d}", [256, M4], BF16) for d in range(2)]
    Whh0T = [din(f"Whh0T{d}", [200, M4], BF16) for d in range(2)]
    W1T = [din(f"W1T{d}", [401, M4], BF16) for d in range(2)]
    Whh1T = [din(f"Whh1T{d}", [200, M4], BF16) for d in range(2)]
    WUs = din("WUs", [401, 800], BF16)
    W2s = din("W2s", [512, 4], BF16)
    tokf = din("tokf", [CHT * BL, NCH], I32)   # [slot(st,lane), chunk]
    tokb = din("tokb", [CHT * BL, NCH], I32)
    uidx0 = din("uidx0", [128, NPT], I32)
    uidx1 = din("uidx1", [128, NPT], I32)
    umask0 = din("umask0", [128, NPT], F32)
    umask1 = din("umask1", [128, NPT], F32)
    bw1m = din("bw1m", [128, 2 * H], BF16)

    OUT = dout("OUT", [NPT * 128, 4], F32)

    # internal DRAM: h^T in t-major order; rows 0:200 fwd, 200:400 bwd, 400 ones
    h0T = nc.dram_tensor("h0T", [401, NSLOT], BF16).ap()
    h1T = nc.dram_tensor("h1T", [401, NSLOT], BF16).ap()
    U0d = nc.dram_tensor("U0d", [NSLOT, 2 * H], BF16).ap()
    U1d = nc.dram_tensor("U1d", [NSLOT, 2 * H], BF16).ap()

    with tile.TileContext(nc) as tc:
        with tc.tile_pool(name="const", bufs=1) as cp, \
             tc.tile_pool(name="state", bufs=1) as sp:

            def load_tiles(src, rows, ncols, pref):
                tiles = []
                r0 = 0
                for h_ in rows:
                    t_ = cp.tile([h_, ncols], BF16, tag=f"{pref}{r0}",
                                 name=f"{pref}{r0}")
                    nc.sync.dma_start(out=t_[:], in_=src[r0:r0 + h_, :])
                    tiles.append(t_)
                    r0 += h_
                return tiles

            W0t = [load_tiles(W0T[d], [128, 128], M4, f"w0{d}") for d in range(2)]
            Whh0t = [load_tiles(Whh0T[d], [128, 72], M4, f"wh0{d}") for d in range(2)]
            W1t = [load_tiles(W1T[d], [128, 128, 128, 17], M4, f"w1{d}") for d in range(2)]
            Whh1t = [load_tiles(Whh1T[d], [128, 72], M4, f"wh1{d}") for d in range(2)]
            WUt = load_tiles(WUs, [128, 128, 128, 17], 800, "wu")
            W2t = load_tiles(W2s, [128, 128, 128, 128], 4, "w2")

            tok_t = [cp.tile([CHT * BL, NCH], I32, tag=f"tok{d}", name=f"tok{d}")
                     for d in range(2)]
            nc.sync.dma_start(out=tok_t[0][:], in_=tokf[:])
            nc.sync.dma_start(out=tok_t[1][:], in_=tokb[:])

            ones_row = cp.tile([1, NSLOT], BF16, name="ones_row")
            nc.vector.memset(ones_row[:], 1.0)
            nc.sync.dma_start(out=h0T[400:401, :], in_=ones_row[:])
            nc.sync.dma_start(out=h1T[400:401, :], in_=ones_row[:])

            ident64 = sp.tile([64, 64], BF16, name="ident64")
            make_identity(nc, ident64[:])
            ident128 = sp.tile([128, 128], BF16, name="ident128")
            make_identity(nc, ident128[:])

            # persistent LSTM state
            gx = [[sp.tile([CHT * BL, 256], BF16, tag=f"gx{d}{p}", name=f"gx{d}{p}")
                   for p in range(2)] for d in range(2)]
            for d in range(2):
                for p in range(2):
                    nc.vector.memset(gx[d][p][:], 0.0)
                    nc.vector.memset(gx[d][p][:, 255:256], 1.0)
            Hh = [[sp.tile([128, GSTEP * 32], BF16, tag=f"Hh{d}{p}", name=f"Hh{d}{p}")
                   for p in range(2)] for d in range(2)]
            cS = [sp.tile([128, 32], F32, tag=f"cS{d}", name=f"cS{d}")
                  for d in range(2)]
            onesb = sp.tile([128, 32], F32, name="onesb")
            nc.vector.memset(onesb[:], 1.0)

            def hslot(d, p):
                return (p % GSTEP) if d == 0 else (GSTEP - 1 - (p % GSTEP))

            def hpar(p):
                return (p // GSTEP) % 2

            # ============ LSTM layers ============
            with tc.tile_pool(name="work", bufs=3) as wp, \
                 tc.tile_pool(name="pg", bufs=2, space="PSUM") as pgp, \
                 tc.tile_pool(name="xp", bufs=2, space="PSUM") as xpp, \
                 tc.tile_pool(name="rhs1", bufs=2) as rp, \
                 tc.tile_pool(name="uw", bufs=2) as uw, \
                 tc.tile_pool(name="ub", bufs=2) as ubp, \
                 tc.tile_pool(name="ups", bufs=1, space="PSUM") as ups:

                xparts = {}
                pg = {}
                rhs1 = {}

                def emit_x0(k):
                    par = k % 2
                    ps = xpp.tile([128, 256], BF16, space="PSUM",
                                  tag="xps", name="xps",
                                  padded_shape=[128, 1024])
                    for d in range(2):
                        gxt = gx[d][par]
                        nc.gpsimd.indirect_dma_start(
                            out=gxt[:, 0:E], out_offset=None, in_=emb[:],
                            in_offset=bass.IndirectOffsetOnAxis(
                                ap=tok_t[d][:, k:k + 1], axis=0))
                        for half in range(2):
                            c0 = (2 * d + half) * 64
                            nc.tensor.transpose(
                                ps[:, c0:c0 + 64],
                                gxt[:, half * 128:(half + 1) * 128],
                                ident64[:])
                            xt = wp.tile([128, 64], BF16, tag=f"x{d}{half}",
                                         name=f"x{d}{half}")
                            nc.vector.tensor_copy(xt[:], ps[:, c0:c0 + 64])
                            xparts[(k, d, half)] = xt

                def emit_xg0(k):
                    for d in range(2):
                        pgt = pgp.tile([128, 512], F32, space="PSUM",
                                       tag=f"PG{d}", name=f"PG{d}")
                        pg[(k, d)] = pgt
                        for kc in range(2):
                            rhs = xparts.pop((k, d, kc))
                            for mc in range(8):
                                nc.tensor.matmul(
                                    pgt[:, mc * 64:(mc + 1) * 64],
                                    W0t[d][kc][:, mc * 128:(mc + 1) * 128],
                                    rhs[:], start=(kc == 0 and mc == 0),
                                    stop=False, skip_group_check=True)

                def load_rhs1(g):
                    for d in range(2):
                        col0 = g * GSTEP * BL if d == 0 else (T - GSTEP * (g + 1)) * BL
                        tiles = []
                        for (r0, r1) in ((0, 128), (128, 256), (256, 384), (384, 401)):
                            t_ = rp.tile([r1 - r0, GSTEP * BL], BF16,
                                         tag=f"R{d}{r0}", name=f"R{d}{r0}")
                            nc.sync.dma_start(
                                out=t_[:], in_=h0T[r0:r1, col0:col0 + GSTEP * BL])
                            tiles.append(t_)
                        rhs1[(g, d)] = tiles

                def emit_xg1(k):
                    g, cg = k // GCH, k % GCH
                    for d in range(2):
                        pgt = pgp.tile([128, 512], F32, space="PSUM",
                                       tag=f"PG{d}", name=f"PG{d}")
                        pg[(k, d)] = pgt
                        tiles = rhs1[(g, d)]
                        coff = cg * 64 if d == 0 else (GSTEP - 4 - 4 * cg) * BL
                        for kc in range(4):
                            rhs = tiles[kc][:, coff:coff + 64]
                            for mc in range(8):
                                nc.tensor.matmul(
                                    pgt[:, mc * 64:(mc + 1) * 64],
                                    W1t[d][kc][:, mc * 128:(mc + 1) * 128],
                                    rhs, start=(kc == 0 and mc == 0),
                                    stop=False, skip_group_check=True)

                def emit_step_dir(k, st, d, Whht, layer):
                    # full per-direction step sequence; f/b emitted alternately
                    # so the two chains phase-shift on the in-order queues
                    p = CHT * k + st
                    q = st if (d == 0 or layer == 0) else CHT - 1 - st
                    pgt = pg[(k, d)]
                    hs = hslot(d, p - 1)
                    hprev = Hh[d][hpar(p - 1)]
                    for kc in range(2):
                        if kc == 0:
                            rhs = hprev[:, hs * 32:hs * 32 + 16]
                        else:
                            rhs = hprev[0:72, hs * 32 + 16:hs * 32 + 32]
                        for mc in range(8):
                            nc.tensor.matmul(
                                pgt[:, mc * 64 + q * 16:mc * 64 + q * 16 + 16],
                                Whht[d][kc][:, mc * 128:(mc + 1) * 128],
                                rhs, start=False, stop=(kc == 1),
                                skip_group_check=True)
                    G = wp.tile([128, 128], F32, tag=f"G{d}", name=f"G{d}")
                    nc.scalar.activation(
                        G[:].rearrange("p (m s) -> p m s", s=16),
                        pgt[:].rearrange("p (m s) -> p m s", s=64)[:, :, q * 16:(q + 1) * 16],
                        AF.Sigmoid)
                    eng = nc.vector
                    dg = wp.tile([128, 32], F32, tag=f"d{d}", name=f"d{d}")
                    eng.scalar_tensor_tensor(
                        dg[:], G[:, 96:128], 2.0, onesb[:], ALU.mult, ALU.subtract)
                    ag = wp.tile([128, 32], F32, tag=f"a{d}", name=f"a{d}")
                    eng.scalar_tensor_tensor(
                        ag[:], G[:, 32:64], 1.0, dg[:], ALU.mult, ALU.mult)
                    Xg = wp.tile([128, 32], F32, tag=f"X{d}", name=f"X{d}")
                    nc.vector.scalar_tensor_tensor(
                        Xg[:], G[:, 0:32], 1.0, cS[d][:], ALU.mult, ALU.mult)
                    nc.vector.scalar_tensor_tensor(
                        cS[d][:], ag[:], 1.0, Xg[:], ALU.mult, ALU.add)
                    TC = wp.tile([128, 32], F32, tag=f"tc{d}", name=f"tc{d}")
                    nc.scalar.activation(TC[:], cS[d][:], AF.Tanh)
                    hsl = hslot(d, p)
                    nc.vector.scalar_tensor_tensor(
                        Hh[d][hpar(p)][:, hsl * 32:(hsl + 1) * 32],
                        G[:, 64:96], 1.0, TC[:], ALU.mult, ALU.mult)

                def emit_step(k, st, Whht, layer):
                    emit_step_dir(k, st, 0, Whht, layer)
                    emit_step_dir(k, st, 1, Whht, layer)

                def store_h(layer, g):
                    hT = h0T if layer == 0 else h1T
                    par = g % 2
                    for d in range(2):
                        rb = 0 if d == 0 else 200
                        tcol0 = g * GSTEP * BL if d == 0 else (T - GSTEP * (g + 1)) * BL
                        src = Hh[d][par]
                        v = src[:].rearrange("p (s c) -> p s c", c=32)
                        v72 = src[0:72, :].rearrange("p (s c) -> p s c", c=32)
                        nc.sync.dma_start(
                            out=hT[rb:rb + 128, tcol0:tcol0 + GSTEP * BL],
                            in_=v[:, :, 0:16])
                        nc.sync.dma_start(
                            out=hT[rb + 128:rb + 200, tcol0:tcol0 + GSTEP * BL],
                            in_=v72[:, :, 16:32])

                # ---- U phase machinery (interleaved into L1's latency gaps)
                UGC = 8                 # chunks (of 128 slots) per U group
                ustate = {"cur": None, "c8": 0, "lt": None, "Ub": None}
                uready = []

                def u_load_group(ug):
                    c0 = ug * UGC * 128
                    lt = []
                    for (r0, r1) in ((0, 128), (128, 256), (256, 384), (384, 401)):
                        t_ = uw.tile([r1 - r0, UGC * 128], BF16, tag=f"ul{r0}",
                                     name=f"ul{r0}")
                        nc.sync.dma_start(out=t_[:], in_=h1T[r0:r1, c0:c0 + UGC * 128])
                        lt.append(t_)
                    Ub = [ubp.tile([128, UGC * 400], BF16, tag=f"Ub{i}",
                                   name=f"Ub{i}") for i in range(2)]
                    ustate.update(cur=ug, c8=0, lt=lt, Ub=Ub)

                def u_emit_chunk():
                    ug, c8 = ustate["cur"], ustate["c8"]
                    lt, Ub = ustate["lt"], ustate["Ub"]
                    psu = ups.tile([128, 800], F32, space="PSUM", tag="psu",
                                   name="psu")
                    for kc in range(4):
                        for ns in range(7):
                            n0, n1 = ns * 128, min((ns + 1) * 128, 800)
                            nc.tensor.matmul(
                                psu[:, n0:n1],
                                lt[kc][:, c8 * 128:(c8 + 1) * 128],
                                WUt[kc][:, n0:n1],
                                start=(kc == 0 and ns in (0, 4)),
                                stop=(kc == 3), skip_group_check=True)
                    nc.vector.tensor_copy(Ub[0][:, c8 * 400:c8 * 400 + 200],
                                          psu[:, 0:200])
                    nc.vector.tensor_copy(Ub[0][:, c8 * 400 + 200:(c8 + 1) * 400],
                                          psu[:, 200:400])
                    nc.scalar.copy(Ub[1][:, c8 * 400:(c8 + 1) * 400],
                                   psu[:, 400:800])
                    if c8 == UGC - 1:
                        c0 = ug * UGC * 128
                        for i, Ud in enumerate((U0d, U1d)):
                            dst = Ud[c0:c0 + UGC * 128, :].rearrange(
                                "(c p) e -> p c e", p=128)
                            nc.sync.dma_start(
                                out=dst,
                                in_=Ub[i][:].rearrange("p (c e) -> p c e", e=400))
                        ustate["cur"] = None
                    else:
                        ustate["c8"] = c8 + 1

                def u_pump(n):
                    for _ in range(n):
                        if ustate["cur"] is None:
                            if not uready:
                                return
                            u_load_group(uready.pop(0))
                        u_emit_chunk()

                def run_layer(layer):
                    Whht = Whh0t if layer == 0 else Whh1t
                    for d in range(2):
                        nc.vector.memset(cS[d][:], 0.0)
                    # zero the h_prev slots read at p=0 (python-mod indices)
                    nc.vector.memset(
                        Hh[0][hpar(-1)][:, hslot(0, -1) * 32:(hslot(0, -1) + 1) * 32], 0.0)
                    nc.vector.memset(
                        Hh[1][hpar(-1)][:, hslot(1, -1) * 32:(hslot(1, -1) + 1) * 32], 0.0)
                    if layer == 0:
                        emit_x0(0)
                        emit_xg0(0)
                    else:
                        load_rhs1(0)
                        emit_xg1(0)
                    for k in range(NCH):
                        if layer == 1 and k % GCH == 8 and k // GCH + 1 < NGRP:
                            load_rhs1(k // GCH + 1)
                        if layer == 0 and k + 1 < NCH:
                            emit_x0(k + 1)
                        emit_step(k, 0, Whht, layer)
                        emit_step(k, 1, Whht, layer)
                        if k + 1 < NCH:
                            if layer == 0:
                                emit_xg0(k + 1)
                            else:
                                emit_xg1(k + 1)
                        emit_step(k, 2, Whht, layer)
                        emit_step(k, 3, Whht, layer)
                        if layer == 1:
                            u_pump(1)
                        if k % GCH == GCH - 1:
                            g = k // GCH
                            store_h(layer, g)
                            if layer == 1 and g >= NGRP // 2:
                                # U group ug needs L1-f group ug and L1-b group
                                # NGRP-1-ug; both done once g >= max(ug, 7-ug)
                                if g == NGRP - 1:
                                    uready.extend([0, NGRP - 1])
                                else:
                                    uready.extend([NGRP - 1 - g, g])

                run_layer(0)
                run_layer(1)
                # drain remaining U work
                u_pump(NGRP * UGC)

            # ============ gather + MLP ============
            with tc.tile_pool(name="fw", bufs=4) as fw, \
                 tc.tile_pool(name="fc", bufs=1) as fc, \
                 tc.tile_pool(name="ob", bufs=2) as obp, \
                 tc.tile_pool(name="fps", bufs=3, space="PSUM") as fps:
                ui0 = fc.tile([128, NPT], I32, name="ui0")
                ui1 = fc.tile([128, NPT], I32, name="ui1")
                um0 = fc.tile([128, NPT], F32, name="um0")
                um1 = fc.tile([128, NPT], F32, name="um1")
                nc.sync.dma_start(out=ui0[:], in_=uidx0[:])
                nc.sync.dma_start(out=ui1[:], in_=uidx1[:])
                nc.sync.dma_start(out=um0[:], in_=umask0[:])
                nc.sync.dma_start(out=um1[:], in_=umask1[:])
                bwt = fc.tile([128, 2 * H], BF16, name="bwt")
                nc.sync.dma_start(out=bwt[:], in_=bw1m[:])
                hm = [fc.tile([128, 512], BF16, tag=f"hm{i}", name=f"hm{i}")
                      for i in range(3)]
                for t_ in hm:
                    nc.vector.memset(t_[:], 0.0)
                    nc.vector.memset(t_[:, 511:512], 1.0)
                OBW = 8
                for j in range(NPT):
                    par = j % 3
                    if j % OBW == 0:
                        ob = obp.tile([128, OBW * 4], F32, tag="ob", name="ob")
                    g0 = fw.tile([128, 2 * H], BF16, tag="g0", name="g0")
                    g1 = fw.tile([128, 2 * H], BF16, tag="g1", name="g1")
                    nc.gpsimd.indirect_dma_start(
                        out=g0[:], out_offset=None, in_=U0d[:],
                        in_offset=bass.IndirectOffsetOnAxis(ap=ui0[:, j:j + 1], axis=0))
                    nc.gpsimd.indirect_dma_start(
                        out=g1[:], out_offset=None, in_=U1d[:],
                        in_offset=bass.IndirectOffsetOnAxis(ap=ui1[:, j:j + 1], axis=0))
                    g1m = fw.tile([128, 2 * H], BF16, tag="g1m", name="g1m")
                    nc.vector.scalar_tensor_tensor(
                        g1m[:], g1[:], um1[:, j:j + 1], bwt[:], ALU.mult, ALU.add)
                    ssum = fw.tile([128, 2 * H], BF16, tag="ssum", name="ssum")
                    nc.vector.scalar_tensor_tensor(
                        ssum[:], g0[:], um0[:, j:j + 1], g1m[:], ALU.mult, ALU.add)
                    nc.scalar.activation(hm[par][:, 0:2 * H], ssum[:], AF.Tanh)
                    hmT = []
                    pst = fps.tile([128, 512], BF16, space="PSUM",
                                   tag="pst", name="pst",
                                   padded_shape=[128, 1024])
                    for i in range(4):
                        nc.tensor.transpose(
                            pst[:, i * 128:(i + 1) * 128],
                            hm[par][:, i * 128:(i + 1) * 128], ident128[:])
                        ht_ = fw.tile([128, 128], BF16, tag=f"hmT{i}", name=f"hmT{i}")
                        nc.vector.tensor_copy(ht_[:], pst[:, i * 128:(i + 1) * 128])
                        hmT.append(ht_)
                    psl = fps.tile([128, 4], F32, space="PSUM", tag="psl",
                                   name="psl", padded_shape=[128, 512])
                    for i in range(4):
                        nc.tensor.matmul(psl[:], hmT[i][:], W2t[i][:],
                                         start=(i == 0), stop=(i == 3))
                    ex = fw.tile([128, 4], F32, tag="ex", name="ex")
                    nc.scalar.activation(ex[:], psl[:], AF.Exp)
                    sm = fw.tile([128, 1], F32, tag="sm", name="sm")
                    nc.vector.reduce_sum(sm[:], ex[:], axis=mybir.AxisListType.X)
                    rc = fw.tile([128, 1], F32, tag="rc", name="rc")
                    nc.vector.reciprocal(rc[:], sm[:])
                    nc.vector.tensor_scalar_mul(
                        ob[:, (j % OBW) * 4:(j % OBW + 1) * 4], ex[:], rc[:, 0:1])
                    if j % OBW == OBW - 1:
                        j0 = j - OBW + 1
                        dst = OUT[j0 * 128:(j0 + OBW) * 128, :].rearrange(
                            "(a p) e -> p a e", p=128)
                        nc.sync.dma_start(
                            out=dst,
                            in_=ob[:].rearrange("p (a e) -> p a e", e=4))
    nc.compile()
    return nc


# ---------------------------------------------------------------------------
# host-side preparation
# ---------------------------------------------------------------------------

def _perm_scale(w):
    """torch gate order (i,f,g,o) -> (f,i,o,g) along axis 0; scale g block x2."""
    Hq = w.shape[0] // 4
    i, f, g, o = (w[0:Hq], w[Hq:2 * Hq], w[2 * Hq:3 * Hq], w[3 * Hq:4 * Hq])
    return np.concatenate([f, i, o, 2.0 * g], axis=0)


def _gatepad(wT):
    """[K, 800] -> [K, 1024] with each 200-row gate block padded to 256."""
    K = wT.shape[0]
    out = np.zeros((K, M4), np.float32)
    for gb in range(4):
        out[:, gb * 256:gb * 256 + 200] = wT[:, gb * 200:(gb + 1) * 200]
    return out


def prepare_inputs(inputs):
    bf = ml_dtypes.bfloat16
    C_ = np.asarray(inputs["confs"]).shape[1]
    emb = np.asarray(inputs["emb"], np.float32)
    tokens = np.asarray(inputs["tokens"])
    confs = np.asarray(inputs["confs"])

    p = {}
    p["emb"] = emb.astype(bf)

    def prep_dir(Wih, Whh, b, kin, kpad, name_w, name_h):
        Wp = _perm_scale(np.asarray(Wih, np.float32))      # [800, kin]
        bp = _perm_scale(np.asarray(b, np.float32))        # [800]
        Hp = _perm_scale(np.asarray(Whh, np.float32))      # [800, 200]
        wt = np.zeros((kpad, M4), np.float32)
        wt[0:kin] = _gatepad(Wp.T)
        wt[kpad - 1] = _gatepad(bp[None, :])[0]
        p[name_w] = wt.astype(bf)
        p[name_h] = _gatepad(Hp.T).astype(bf)

    prep_dir(inputs["Wih0f"], inputs["Whh0f"], inputs["b0f"], E, 256, "W0T0", "Whh0T0")
    prep_dir(inputs["Wih0b"], inputs["Whh0b"], inputs["b0b"], E, 256, "W0T1", "Whh0T1")
    prep_dir(inputs["Wih1f"], inputs["Whh1f"], inputs["b1f"], 400, 401, "W1T0", "Whh1T0")
    prep_dir(inputs["Wih1b"], inputs["Whh1b"], inputs["b1b"], 400, 401, "W1T1", "Whh1T1")

    w1 = np.asarray(inputs["w1"], np.float32)
    bw1 = np.asarray(inputs["bw1"], np.float32)
    w2 = np.asarray(inputs["w2"], np.float32)
    bw2 = np.asarray(inputs["bw2"], np.float32)

    wu = np.zeros((401, 800), np.float32)
    wu[0:400, 0:400] = w1[:, 0:400].T
    wu[0:400, 400:800] = w1[:, 400:800].T
    p["WUs"] = wu.astype(bf)
    p["bw1m"] = np.tile(bw1[None, :], (128, 1)).astype(bf)
    w2p = np.zeros((512, 4), np.float32)
    w2p[0:400] = w2.T
    w2p[511] = bw2
    p["W2s"] = w2p.astype(bf)

    NP = BL * C_
    NPT = (NP + 127) // 128

    in_maps = []
    for c in range(NCORE):
        m = dict(p)
        bs = tokens[c * BL:(c + 1) * BL, 0:T]          # [BL, T]
        tf = np.zeros((CHT * BL, NCH), np.int32)
        tb = np.zeros((CHT * BL, NCH), np.int32)
        for k in range(NCH):
            for tr in range(CHT):
                tf[tr * BL:(tr + 1) * BL, k] = bs[:, k * CHT + tr]
                tb[tr * BL:(tr + 1) * BL, k] = bs[:, T - 1 - (k * CHT + tr)]
        m["tokf"] = tf
        m["tokb"] = tb
        cf = confs[c * BL:(c + 1) * BL]                 # [BL, C, 2]
        t0 = cf[:, :, 0].reshape(-1)
        t1 = cf[:, :, 1].reshape(-1)
        bidx = np.repeat(np.arange(BL), C_)
        ui0 = np.clip(t0, 0, T - 1) * BL + bidx
        ui1 = np.clip(t1, 0, T - 1) * BL + bidx
        um0 = (t0 >= 0).astype(np.float32)
        um1 = (t1 >= 0).astype(np.float32)

        def tile128(a, dt):
            o = np.zeros((NPT * 128,), dt)
            o[:a.shape[0]] = a
            return o.reshape(NPT, 128).T.copy()
        m["uidx0"] = tile128(ui0.astype(np.int32), np.int32)
        m["uidx1"] = tile128(ui1.astype(np.int32), np.int32)
        m["umask0"] = tile128(um0, np.float32)
        m["umask1"] = tile128(um1, np.float32)
        in_maps.append(m)
    return in_maps


_CACHE = {}


def _get_prog(NPT):
    if NPT not in _CACHE:
        _CACHE[NPT] = build(NPT)
    return _CACHE[NPT]


def kernel(**inputs):
    C_ = inputs["confs"].shape[1]
    NP = BL * C_
    NPT = (NP + 127) // 128
    nc = _get_prog(NPT)
    in_maps = prepare_inputs(inputs)
    res = run_bass_kernel_spmd(nc, in_maps, list(range(NCORE)))
    outs = []
    for c in range(NCORE):
        o = res.results[c]["OUT"][:NP]
        outs.append(o)
    return np.concatenate(outs, axis=0).astype(np.float32)
